# revision 1
# baseline (speedup 1.0000x reference)
"""Trainium2 Bass kernel for nn_Attention_71846212928150.

Self-attention block (pre-LN + silu, QKV projections, per-head attention with
q/k LayerNorms, output projection), sharded over 8 NeuronCores by heads:
core c owns heads {2c, 2c+1} = inner columns [128c, 128c+128).

Per core:
  phase 1: stream x in 128-token tiles; LN (bn_stats + quake-rsqrt on DVE,
           no ACT table switches) fused with silu on ACT; PE-transpose;
           fused QKV matmul (fp32r) into [tok, 384] PSUM; evict q/k/v.
  phase 2: partial sum / sumsq of q,k over the local 128 columns;
           AllReduce [128,128] stats across the 8 cores (full 1024-wide LN).
  phase 3: apply q/k LayerNorm in [tok, col] layout (per-partition scalars),
           PE-transpose to [col, tok], apply gain/bias (+ inner**-0.5 folded
           into the q gain on host).
  phase 4: per (batch, head): S^T = K^T.T @ q^T tiles -> exp on ACT (no max
           subtraction: |scores| <~ 1.5 by construction) -> PV matmul with a
           ones-column appended to V so PSUM row 64 accumulates the softmax
           denominator; normalize via reciprocal + PE broadcast.
  phase 5: silu(O) and output projection -> out^T [1024, 4096] partial sums,
           host adds the 8 partials, transposes, adds b_o.
"""

import numpy as np

import concourse.bass as bass
import concourse.mybir as mybir
import concourse.tile as tile
from concourse.masks import make_identity

F32 = mybir.dt.float32
F32R = mybir.dt.float32r
I32 = mybir.dt.int32
AF = mybir.ActivationFunctionType
ALU = mybir.AluOpType
AX = mybir.AxisListType

B = 2
C = 1024
H = 16
DH = 64
INNER = H * DH
NCORES = 8
HL = H // NCORES          # 2 heads per core
CL = HL * DH              # 128 local inner columns
QKV = 3 * CL              # 384
KT = C // 128             # 8 contraction tiles over C
EPS = 1e-5
MAGIC = 0x5F3759DF


def _quake_rsqrt(nc, pool, vpe, shape, suffix=""):
    """rstd = 1/sqrt(vpe) entirely on DVE (fp32-exact after 3 Newton steps)."""
    y = pool.tile(list(shape), F32, name=f"qk_y{suffix}")
    t2 = pool.tile(list(shape), F32, name=f"qk_t2{suffix}")
    nc.vector.tensor_scalar(
        out=y.bitcast(I32), in0=vpe.bitcast(I32), scalar1=1, scalar2=None,
        op0=ALU.logical_shift_right)
    nc.vector.tensor_scalar(
        out=y.bitcast(I32), in0=y.bitcast(I32), scalar1=-1, scalar2=MAGIC,
        op0=ALU.mult, op1=ALU.add)
    for _ in range(3):
        nc.vector.tensor_tensor(out=t2, in0=y, in1=y, op=ALU.mult)
        nc.vector.tensor_tensor(out=t2, in0=t2, in1=vpe, op=ALU.mult)
        nc.vector.tensor_scalar(out=t2, in0=t2, scalar1=-0.5, scalar2=1.5,
                                op0=ALU.mult, op1=ALU.add)
        nc.vector.tensor_tensor(out=y, in0=y, in1=t2, op=ALU.mult)
    return y


def _quake_rsqrt2(nc, pool, vpe, shape, suffix=""):
    """Two-iteration variant (~4e-6 rel err) for the latency-critical x path."""
    y = pool.tile(list(shape), F32, name=f"qj_y{suffix}")
    t2 = pool.tile(list(shape), F32, name=f"qj_t2{suffix}")
    nc.vector.tensor_scalar(
        out=y.bitcast(I32), in0=vpe.bitcast(I32), scalar1=1, scalar2=None,
        op0=ALU.logical_shift_right)
    nc.vector.tensor_scalar(
        out=y.bitcast(I32), in0=y.bitcast(I32), scalar1=-1, scalar2=MAGIC,
        op0=ALU.mult, op1=ALU.add)
    for _ in range(2):
        nc.vector.tensor_tensor(out=t2, in0=y, in1=y, op=ALU.mult)
        nc.vector.tensor_tensor(out=t2, in0=t2, in1=vpe, op=ALU.mult)
        nc.vector.tensor_scalar(out=t2, in0=t2, scalar1=-0.5, scalar2=1.5,
                                op0=ALU.mult, op1=ALU.add)
        nc.vector.tensor_tensor(out=y, in0=y, in1=t2, op=ALU.mult)
    return y


def _fixup_module(nc):
    """Adapt Tile-emitted BIR to this container's walrus build.

    1. The tail `EVENT_SEMAPHORE_RANGE_CLEAR` InstISA (opcode 176) is not
       understood by this walrus' birverifier. Replace it with one
       EventSemaphore sem-write-0 per semaphore in the cleared range
       (functionally equivalent, re-execution stays safe).
    2. Drain instructions carrying more than one semaphore wait fail codegen
       ("Too many sync wait commands"). Hoist the extra waits into standalone
       EventSemaphore wait instructions just before the drain.
    """
    for f in nc.m.functions:
        for bb in f.blocks:
            newlist = []
            changed = False
            for ins in bb.instructions:
                tn = type(ins).__name__
                if tn == "InstISA" and getattr(ins, "isa_opcode", None) == 176:
                    ad = ins.ant_dict or {}
                    first = ad.get("range_first")
                    last = ad.get("range_last")
                    if first is not None and last is not None:
                        si = ins.sync_info
                        sems = list(range(first, last + 1))
                        for k, sem in enumerate(sems):
                            ev = mybir.InstEventSemaphore(
                                name=f"{ins.name}-clr{k}", engine=ins.engine,
                                ins=[], outs=[])
                            upd = mybir.SyncUpdate(
                                sync_type="semaphore", id=sem,
                                update_mode="sem-wr-imm", update_value=0)
                            on_wait = (list(si.on_wait)
                                       if (k == 0 and si is not None and si.on_wait)
                                       else [])
                            ev.sync_info = mybir.SyncInfo(
                                on_wait=on_wait, on_update=[upd])
                            newlist.append(ev)
                        if si is not None and si.on_update:
                            evf = mybir.InstEventSemaphore(
                                name=f"{ins.name}-clrf", engine=ins.engine,
                                ins=[], outs=[])
                            evf.sync_info = mybir.SyncInfo(
                                on_wait=[], on_update=list(si.on_update))
                            newlist.append(evf)
                    changed = True
                    continue
                si = ins.sync_info
                if (si is not None and si.on_wait is not None
                        and len(si.on_wait) > 1):
                    waits = list(si.on_wait)
                    for i, w in enumerate(waits[1:]):
                        ev = mybir.InstEventSemaphore(
                            name=f"{ins.name}-hw{i}", engine=ins.engine,
                            ins=[], outs=[])
                        ev.sync_info = mybir.SyncInfo(on_wait=[w], on_update=[])
                        newlist.append(ev)
                    si.on_wait = [waits[0]]
                    ins.sync_info = si
                    changed = True
                newlist.append(ins)
            if changed:
                bb.instructions = newlist
    return nc


def build_bass(n_tok_per_batch, n_cores=NCORES):
    N = n_tok_per_batch
    T = B * N
    NT = T // 128             # token tiles
    KB = N // 128             # key tiles per batch
    QC = max(1, N // 512)     # q chunks per batch
    QCW = min(512, N)         # q chunk width
    OTC = max(1, T // 512)    # out-proj token chunks
    OTW = min(512, T)

    nc = bass.Bass(trn_type="TRN2", num_devices=n_cores)

    x = nc.dram_tensor("x", [T, C], F32, kind="ExternalInput")
    w_all = nc.dram_tensor("w_all", [C, QKV], F32R, kind="ExternalInput")
    b_all = nc.dram_tensor("b_all", [1, QKV], F32, kind="ExternalInput")
    gbe = nc.dram_tensor("gbe", [128, 4], F32, kind="ExternalInput")
    w_o_loc = nc.dram_tensor("w_o_loc", [CL, C], F32R, kind="ExternalInput")
    out_t = nc.dram_tensor("out_t", [C, T], F32, kind="ExternalOutput")

    with tile.TileContext(nc) as tc:
        _body(tc, x, w_all, b_all, gbe, w_o_loc, out_t,
              N=N, T=T, NT=NT, KB=KB, QC=QC, QCW=QCW, OTC=OTC, OTW=OTW,
              n_cores=n_cores)
    return _fixup_module(nc)


def _body(tc, x, w_all, b_all, gbe, w_o_loc, out_t,
          N, T, NT, KB, QC, QCW, OTC, OTW, n_cores):
    nc = tc.nc

    from contextlib import ExitStack
    octx = ExitStack()
    persist = octx.enter_context(tc.tile_pool(name="persist", bufs=1))

    ident = persist.tile([128, 128], F32)
    make_identity(nc, ident)

    w_all_sb = persist.tile([128, KT, QKV], F32R)
    for kt in range(KT):
        nc.sync.dma_start(out=w_all_sb[:, kt, :],
                          in_=w_all[kt * 128:(kt + 1) * 128, :])
    b_sb = persist.tile([128, QKV], F32)
    nc.sync.dma_start(out=b_sb, in_=b_all.ap().to_broadcast([128, QKV]))
    gbe_sb = persist.tile([128, 4], F32)
    nc.sync.dma_start(out=gbe_sb, in_=gbe[:, :])
    w_o_sb = persist.tile([128, C], F32R)
    nc.sync.dma_start(out=w_o_sb, in_=w_o_loc[:, :])

    qT = persist.tile([128, T], F32R)      # [local col, token]
    kTt = persist.tile([128, T], F32R)
    v_aug = persist.tile([128, NT, 130], F32R)   # [tok%128, tile, head-block]
    q_pre = persist.tile([128, NT, 128], F32)   # [tok%128, tile, local col]
    k_pre = persist.tile([128, NT, 128], F32)
    stats = persist.tile([128, 4 * NT], F32)
    stats_all = persist.tile([128, 4 * NT], F32)

    ones_col = persist.tile([128, NT], F32)
    nc.vector.memset(ones_col, 1.0)
    nc.vector.tensor_copy(out=v_aug[:, :, 64:65], in_=ones_col)
    nc.vector.tensor_copy(out=v_aug[:, :, 129:130], in_=ones_col)

    # ---------------- phase 1: x-side LN+silu, transpose, QKV ----------------
    GB = 4  # token tiles per group
    with tc.tile_pool(name="ph1", bufs=3) as ph1, \
         tc.tile_pool(name="ph1s", bufs=4) as ph1s, \
         tc.tile_pool(name="ph1p", bufs=2, space="PSUM") as ph1p, \
         tc.tile_pool(name="ph1q", bufs=3, space="PSUM") as ph1q:
        for g in range(NT // GB):
            xg = ph1.tile([128, GB, C], F32, name="xg")
            nc.sync.dma_start(
                out=xg,
                in_=x[g * GB * 128:(g + 1) * GB * 128, :].rearrange(
                    "(t p) c -> p t c", p=128))

            stats6 = ph1s.tile([128, GB, 2, 6], F32, name="stats6")
            for t in range(GB):
                for h2 in range(2):
                    nc.vector.bn_stats(out=stats6[:, t, h2, :],
                                       in_=xg[:, t, h2 * 512:(h2 + 1) * 512])
            mv = ph1s.tile([128, GB, 2], F32, name="mv")
            for t in range(GB):
                nc.vector.bn_aggr(out=mv[:, t, :], in_=stats6[:, t, :, :])

            vpe = ph1s.tile([128, GB, 1], F32, name="vpe")
            nc.vector.tensor_scalar(out=vpe, in0=mv[:, :, 1:2], scalar1=EPS,
                                    scalar2=None, op0=ALU.add)
            rstd = _quake_rsqrt2(nc, ph1s, vpe, (128, GB, 1))
            nmr = ph1s.tile([128, GB, 1], F32, name="nmr")
            nc.vector.tensor_tensor(out=nmr, in0=mv[:, :, 0:1], in1=rstd,
                                    op=ALU.mult)
            nc.vector.tensor_scalar(out=nmr, in0=nmr, scalar1=-1.0,
                                    scalar2=None, op0=ALU.mult)

            # silu(LN(x)) in place to keep SBUF within budget
            for t in range(GB):
                nc.scalar.activation(out=xg[:, t, :], in_=xg[:, t, :],
                                     func=AF.Silu,
                                     bias=nmr[:, t, :],
                                     scale=rstd[:, t, :])

            for t in range(GB):
                tt = g * GB + t
                pxT = ph1p.tile([128, 1024], F32, name="pxT")
                for j in range(KT):
                    nc.tensor.transpose(pxT[:, j * 128:(j + 1) * 128],
                                        xg[:, t, j * 128:(j + 1) * 128],
                                        ident)
                xsT = ph1.tile([128, 1024], F32R, name="xsT")
                if t % 2 == 0:
                    nc.vector.tensor_copy(out=xsT, in_=pxT)
                else:
                    nc.scalar.copy(out=xsT, in_=pxT)

                pqkv = ph1q.tile([128, 512], F32, name="pqkv")
                for kt in range(KT):
                    nc.tensor.matmul(
                        pqkv[:, 0:QKV],
                        lhsT=xsT[:, kt * 128:(kt + 1) * 128],
                        rhs=w_all_sb[:, kt, :],
                        start=(kt == 0), stop=(kt == KT - 1))

                nc.vector.scalar_tensor_tensor(
                    out=q_pre[:, tt, :], in0=pqkv[:, 0:128], scalar=1.0,
                    in1=b_sb[:, 0:128], op0=ALU.mult, op1=ALU.add)
                nc.vector.scalar_tensor_tensor(
                    out=k_pre[:, tt, :], in0=pqkv[:, 128:256], scalar=1.0,
                    in1=b_sb[:, 128:256], op0=ALU.mult, op1=ALU.add)
                nc.vector.scalar_tensor_tensor(
                    out=v_aug[:, tt, :].rearrange("p (h e) -> p h e", e=65)[:, :, 0:64],
                    in0=pqkv[:, 256:384].rearrange("p (h e) -> p h e", e=64),
                    scalar=1.0,
                    in1=b_sb[:, 256:384].rearrange("p (h e) -> p h e", e=64),
                    op0=ALU.mult, op1=ALU.add)

    # ---------------- phase 2: q/k stats + AllReduce ----------------
    with tc.tile_pool(name="ph2", bufs=1) as ph2:
        nc.vector.tensor_reduce(out=stats[:, 0:NT], in_=q_pre[:, :, :],
                                axis=AX.X, op=ALU.add)
        nc.vector.tensor_reduce(out=stats[:, 2 * NT:3 * NT], in_=k_pre[:, :, :],
                                axis=AX.X, op=ALU.add)
        scr = ph2.tile([128, 128], F32, name="scr")
        for tt in range(NT):
            nc.scalar.activation(
                out=scr, in_=q_pre[:, tt, :], func=AF.Square,
                accum_out=stats[:, NT + tt:NT + tt + 1])
        for tt in range(NT):
            nc.scalar.activation(
                out=scr, in_=k_pre[:, tt, :], func=AF.Square,
                accum_out=stats[:, 3 * NT + tt:3 * NT + tt + 1])

        with tc.tile_pool(name="dram", bufs=1, space="DRAM") as dpool:
            cc_in = dpool.tile([128, 4 * NT], F32, name="cc_in")
            cc_out = dpool.tile([128, 4 * NT], F32, name="cc_out",
                                addr_space="Shared")
            nc.sync.dma_start(out=cc_in, in_=stats)
            nc.gpsimd.collective_compute(
                "AllReduce", ALU.add,
                replica_groups=[list(range(n_cores))],
                ins=[cc_in.opt()], outs=[cc_out.opt()])
            nc.sync.dma_start(out=stats_all, in_=cc_out)

        # per-token mean / rstd for q and k (over full 1024-wide inner dim)
        qk_stats = []
        for which in range(2):  # 0 -> q, 1 -> k
            s_sum = stats_all[:, 2 * which * NT:(2 * which + 1) * NT]
            s_ssq = stats_all[:, (2 * which + 1) * NT:(2 * which + 2) * NT]
            m = ph2.tile([128, NT], F32, name=f"m_{which}")
            nc.vector.tensor_scalar(out=m, in0=s_sum, scalar1=1.0 / INNER,
                                    scalar2=None, op0=ALU.mult)
            msq = ph2.tile([128, NT], F32, name=f"msq_{which}")
            nc.vector.tensor_scalar(out=msq, in0=s_ssq, scalar1=1.0 / INNER,
                                    scalar2=None, op0=ALU.mult)
            tmp = ph2.tile([128, NT], F32, name=f"tmp_{which}")
            nc.vector.tensor_tensor(out=tmp, in0=m, in1=m, op=ALU.mult)
            nc.vector.tensor_tensor(out=tmp, in0=msq, in1=tmp, op=ALU.subtract)
            nc.vector.tensor_scalar(out=tmp, in0=tmp, scalar1=EPS,
                                    scalar2=None, op0=ALU.add)
            rstd = _quake_rsqrt(nc, ph2, tmp, (128, NT), suffix=f"_{which}")
            nmr = ph2.tile([128, NT], F32, name=f"nmr_{which}")
            nc.vector.tensor_tensor(out=nmr, in0=m, in1=rstd, op=ALU.mult)
            nc.vector.tensor_scalar(out=nmr, in0=nmr, scalar1=-1.0,
                                    scalar2=None, op0=ALU.mult)
            qk_stats.append((m, rstd, nmr))

        # ---------------- phase 3: apply LN, transpose q/k ----------------
        with tc.tile_pool(name="ph3", bufs=8) as ph3, \
             tc.tile_pool(name="ph3p", bufs=4, space="PSUM") as ph3p:
            for which, (pre, dst, gcol) in enumerate(
                    [(q_pre, qT, 0), (k_pre, kTt, 2)]):
                m, rstd, nmr = qk_stats[which]
                for tt in range(NT):
                    qn = ph3.tile([128, 128], F32, name="qn")
                    if which == 1:
                        # k: normalize on ACT so DVE and ACT each carry one
                        # of the two per-tile passes
                        nc.scalar.activation(
                            out=qn, in_=pre[:, tt, :], func=AF.Identity,
                            bias=nmr[:, tt:tt + 1],
                            scale=rstd[:, tt:tt + 1])
                    else:
                        nc.vector.tensor_scalar(
                            out=qn, in0=pre[:, tt, :],
                            scalar1=m[:, tt:tt + 1],
                            scalar2=rstd[:, tt:tt + 1],
                            op0=ALU.subtract, op1=ALU.mult)
                    pq = ph3p.tile([128, 128], F32, name="pq")
                    nc.tensor.transpose(pq, qn, ident)
                    if which == 0:
                        nc.scalar.activation(
                            out=dst[:, tt * 128:(tt + 1) * 128], in_=pq,
                            func=AF.Identity,
                            bias=gbe_sb[:, gcol + 1:gcol + 2],
                            scale=gbe_sb[:, gcol:gcol + 1])
                    else:
                        nc.vector.tensor_scalar(
                            out=dst[:, tt * 128:(tt + 1) * 128], in0=pq,
                            scalar1=gbe_sb[:, gcol:gcol + 1],
                            scalar2=gbe_sb[:, gcol + 1:gcol + 2],
                            op0=ALU.mult, op1=ALU.add)

    # ---------------- phase 4: attention ----------------
    att45 = octx.enter_context(tc.tile_pool(name="att45", bufs=1))
    onorm = att45.tile([128, T], F32)
    siluo = att45.tile([128, T], F32R)
    with tc.tile_pool(name="att", bufs=3) as att, \
         tc.tile_pool(name="dramsc", bufs=2, space="DRAM") as dramsc, \
         tc.tile_pool(name="attp", bufs=2, space="PSUM") as attp, \
         tc.tile_pool(name="attpo", bufs=1, space="PSUM") as attpo:
        NHALF = min(2, QC)            # chunk groups per key tile
        HC = QC // NHALF              # q chunks per group
        for b in range(B):
            for h in range(HL):
                pO = attpo.tile([128, QC, QCW], F32, name="pO", tag="pO")
                for kb in range(KB):
                    vt = b * KB + kb
                    for g in range(NHALF):
                        # two [128, HC*QCW] score tiles double-buffer so the
                        # S matmuls of the next group overlap this group's exp
                        pS = attp.tile([128, HC * QCW], F32, name="pS",
                                       tag="pS")
                        for qi in range(HC):
                            qc = g * HC + qi
                            nc.tensor.matmul(
                                pS[:, qi * QCW:(qi + 1) * QCW],
                                lhsT=kTt[h * 64:(h + 1) * 64,
                                         b * N + kb * 128:
                                         b * N + (kb + 1) * 128],
                                rhs=qT[h * 64:(h + 1) * 64,
                                       b * N + qc * QCW:
                                       b * N + (qc + 1) * QCW],
                                start=True, stop=True)
                        eS = att.tile([128, HC * QCW], F32R, name="eS")
                        nc.scalar.activation(out=eS, in_=pS, func=AF.Exp)
                        for qi in range(HC):
                            qc = g * HC + qi
                            nc.tensor.matmul(
                                pO[0:65, qc, :],
                                lhsT=v_aug[:, vt, h * 65:(h + 1) * 65],
                                rhs=eS[:, qi * QCW:(qi + 1) * QCW],
                                start=(kb == 0), stop=(kb == KB - 1))

                dn = att.tile([1, QC, QCW], F32, name="dn")
                nc.vector.reciprocal(out=dn, in_=pO[64:65, :, :])
                dn_dram = dramsc.tile([1, QC, QCW], F32, name="dn_dram")
                nc.sync.dma_start(out=dn_dram, in_=dn)
                dnb = att.tile([64, QC, QCW], F32, name="dnb")
                nc.sync.dma_start(out=dnb, in_=dn_dram.to_broadcast([64, QC, QCW]))
                nc.vector.tensor_tensor(
                    out=onorm[h * 64:(h + 1) * 64, b * N:(b + 1) * N],
                    in0=pO[0:64, :, :], in1=dnb, op=ALU.mult)

    # ---------------- phase 5: silu(O) + output projection ----------------
    with tc.tile_pool(name="ph5", bufs=4) as ph5, \
         tc.tile_pool(name="ph5p", bufs=4, space="PSUM") as ph5p:
        for half in range(max(1, T // 2048)):
            w = min(2048, T)
            nc.scalar.activation(out=siluo[:, half * w:(half + 1) * w],
                                 in_=onorm[:, half * w:(half + 1) * w],
                                 func=AF.Silu)
        for ct in range(KT):
            for tk in range(OTC):
                po = ph5p.tile([128, OTW], F32, name="po")
                nc.tensor.matmul(
                    po,
                    lhsT=w_o_sb[:, ct * 128:(ct + 1) * 128],
                    rhs=siluo[:, tk * OTW:(tk + 1) * OTW],
                    start=True, stop=True)
                ev = ph5.tile([128, OTW], F32, name="ev")
                if (ct * OTC + tk) % 2 == 0:
                    nc.vector.tensor_copy(out=ev, in_=po)
                else:
                    nc.scalar.copy(out=ev, in_=po)
                nc.sync.dma_start(
                    out=out_t[ct * 128:(ct + 1) * 128,
                              tk * OTW:(tk + 1) * OTW],
                    in_=ev)

    octx.close()


def make_in_maps(inputs, n_tok_per_batch, n_cores=NCORES):
    """Slice full inputs into per-core input maps (head sharding)."""
    x = np.ascontiguousarray(np.asarray(inputs["x"], np.float32)
                             .reshape(B * n_tok_per_batch, C))
    w_q = np.asarray(inputs["w_q"], np.float32)
    w_k = np.asarray(inputs["w_k"], np.float32)
    w_v = np.asarray(inputs["w_v"], np.float32)
    b_q = np.asarray(inputs["b_q"], np.float32)
    b_k = np.asarray(inputs["b_k"], np.float32)
    b_v = np.asarray(inputs["b_v"], np.float32)
    g_q = np.asarray(inputs["g_q"], np.float32)
    be_q = np.asarray(inputs["be_q"], np.float32)
    g_k = np.asarray(inputs["g_k"], np.float32)
    be_k = np.asarray(inputs["be_k"], np.float32)
    w_o = np.asarray(inputs["w_o"], np.float32)

    scale = float(INNER) ** -0.5
    in_maps = []
    for c in range(n_cores):
        cols = slice(c * CL, (c + 1) * CL)
        w_all = np.ascontiguousarray(
            np.concatenate([w_q[:, cols], w_k[:, cols], w_v[:, cols]], axis=1))
        b_all = np.ascontiguousarray(
            np.concatenate([b_q[cols], b_k[cols], b_v[cols]])[None, :])
        gbe = np.ascontiguousarray(
            np.stack([g_q[cols] * scale, be_q[cols] * scale,
                      g_k[cols], be_k[cols]], axis=1))
        w_o_c = np.ascontiguousarray(w_o[cols, :])
        in_maps.append({
            "x": x, "w_all": w_all, "b_all": b_all,
            "gbe": gbe, "w_o_loc": w_o_c,
        })
    return in_maps


def combine_outputs(out_ts, inputs, n_tok_per_batch):
    b_o = np.asarray(inputs["b_o"], np.float32)
    acc = np.zeros_like(out_ts[0], dtype=np.float64)
    for o in out_ts:
        acc += o.astype(np.float64)
    out = acc.T.astype(np.float32) + b_o[None, :]
    return out.reshape(B, n_tok_per_batch, C).astype(np.float32)


_NC_CACHE = {}


def kernel(**inputs):
    from concourse.bass_utils import run_bass_kernel_spmd

    n_tok = np.asarray(inputs["x"]).shape[1]
    if n_tok not in _NC_CACHE:
        _NC_CACHE[n_tok] = build_bass(n_tok)
    nc = _NC_CACHE[n_tok]
    in_maps = make_in_maps(inputs, n_tok)
    res = run_bass_kernel_spmd(nc, in_maps, core_ids=list(range(NCORES)))
    out_ts = [r["out_t"] for r in res.results]
    return combine_outputs(out_ts, inputs, n_tok)



# revision 6
# speedup vs baseline: 1.0837x; 1.0837x over previous
"""Trainium2 Bass kernel for nn_Attention_71846212928150.

Self-attention block (pre-LN + silu, QKV projections, per-head attention with
q/k LayerNorms, output projection), sharded over 8 NeuronCores by heads:
core c owns heads {2c, 2c+1} = inner columns [128c, 128c+128).

v2: bf16 matmul datapath + head-interleaved attention.
  phase 1: stream x in 128-token tiles; LN (bn_stats + quake-rsqrt on DVE)
           fused with silu on ACT emitting bf16; PE-transpose (bf16);
           fused QKV matmul (bf16) into [tok, 384] PSUM; evict q/k/v bf16.
  phase 2: partial sum / sumsq of q,k over the local 128 columns;
           AllReduce [128,128] f32 stats across the 8 cores.
  phase 3: apply q/k LayerNorm in [tok, col] layout, PE-transpose to
           [col, tok] bf16, apply gain/bias (inner**-0.5 folded into q gain).
  phase 4: per (batch, q-chunk): both heads interleaved. S matmuls have K=64
           and per-head base partitions 0/64, so the two heads' S matmuls run
           CONCURRENTLY on disjoint PE row-groups (auto tile_position).
           exp on ACT (no max subtraction; |scores| small by construction)
           emits bf16; PV matmul with ones-column in V accumulates the
           softmax denominator in PSUM row 64.
  phase 4b: batched denominator reciprocal: gather all [1,512] denominator
           rows to DRAM, reload as [128,64], one DVE reciprocal, scatter-
           broadcast back to [128, T] via DMA; normalize + silu in two
           full-width passes.
  phase 5: output projection (bf16) -> out^T [1024, 4096] f32 partial sums,
           host adds the 8 partials, transposes, adds b_o.
"""

import numpy as np

import concourse.bass as bass
import concourse.mybir as mybir
import concourse.tile as tile
from concourse.masks import make_identity

F32 = mybir.dt.float32
BF16 = mybir.dt.bfloat16
I32 = mybir.dt.int32
AF = mybir.ActivationFunctionType
ALU = mybir.AluOpType
AX = mybir.AxisListType

B = 2
C = 1024
H = 16
DH = 64
INNER = H * DH
NCORES = 8
HL = H // NCORES          # 2 heads per core
CL = HL * DH              # 128 local inner columns
QKV = 3 * CL              # 384
KT = C // 128             # 8 contraction tiles over C
EPS = 1e-5
MAGIC = 0x5F3759DF
QW = 512                  # attention q-chunk width


def _quake_rsqrt(nc, pool, vpe, shape, suffix=""):
    """rstd = 1/sqrt(vpe) entirely on DVE (fp32-exact after 3 Newton steps)."""
    y = pool.tile(list(shape), F32, name=f"qk_y{suffix}")
    t2 = pool.tile(list(shape), F32, name=f"qk_t2{suffix}")
    nc.vector.tensor_scalar(
        out=y.bitcast(I32), in0=vpe.bitcast(I32), scalar1=1, scalar2=None,
        op0=ALU.logical_shift_right)
    nc.vector.tensor_scalar(
        out=y.bitcast(I32), in0=y.bitcast(I32), scalar1=-1, scalar2=MAGIC,
        op0=ALU.mult, op1=ALU.add)
    for _ in range(3):
        nc.vector.tensor_tensor(out=t2, in0=y, in1=y, op=ALU.mult)
        nc.vector.tensor_tensor(out=t2, in0=t2, in1=vpe, op=ALU.mult)
        nc.vector.tensor_scalar(out=t2, in0=t2, scalar1=-0.5, scalar2=1.5,
                                op0=ALU.mult, op1=ALU.add)
        nc.vector.tensor_tensor(out=y, in0=y, in1=t2, op=ALU.mult)
    return y


def _quake_rsqrt2(nc, pool, vpe, shape, suffix=""):
    """Two-iteration variant (~4e-6 rel err) for the latency-critical x path."""
    y = pool.tile(list(shape), F32, name=f"qj_y{suffix}")
    t2 = pool.tile(list(shape), F32, name=f"qj_t2{suffix}")
    nc.vector.tensor_scalar(
        out=y.bitcast(I32), in0=vpe.bitcast(I32), scalar1=1, scalar2=None,
        op0=ALU.logical_shift_right)
    nc.vector.tensor_scalar(
        out=y.bitcast(I32), in0=y.bitcast(I32), scalar1=-1, scalar2=MAGIC,
        op0=ALU.mult, op1=ALU.add)
    for _ in range(2):
        nc.vector.tensor_tensor(out=t2, in0=y, in1=y, op=ALU.mult)
        nc.vector.tensor_tensor(out=t2, in0=t2, in1=vpe, op=ALU.mult)
        nc.vector.tensor_scalar(out=t2, in0=t2, scalar1=-0.5, scalar2=1.5,
                                op0=ALU.mult, op1=ALU.add)
        nc.vector.tensor_tensor(out=y, in0=y, in1=t2, op=ALU.mult)
    return y


def _fixup_module(nc):
    """Adapt Tile-emitted BIR to this container's walrus build.

    1. The tail `EVENT_SEMAPHORE_RANGE_CLEAR` InstISA (opcode 176) is not
       understood by this walrus' birverifier. Replace it with one
       EventSemaphore sem-write-0 per semaphore in the cleared range
       (functionally equivalent, re-execution stays safe).
    2. Drain instructions carrying more than one semaphore wait fail codegen
       ("Too many sync wait commands"). Hoist the extra waits into standalone
       EventSemaphore wait instructions just before the drain.
    """
    for f in nc.m.functions:
        for bb in f.blocks:
            newlist = []
            changed = False
            for ins in bb.instructions:
                tn = type(ins).__name__
                if tn == "InstISA" and getattr(ins, "isa_opcode", None) == 176:
                    ad = ins.ant_dict or {}
                    first = ad.get("range_first")
                    last = ad.get("range_last")
                    if first is not None and last is not None:
                        si = ins.sync_info
                        sems = list(range(first, last + 1))
                        for k, sem in enumerate(sems):
                            ev = mybir.InstEventSemaphore(
                                name=f"{ins.name}-clr{k}", engine=ins.engine,
                                ins=[], outs=[])
                            upd = mybir.SyncUpdate(
                                sync_type="semaphore", id=sem,
                                update_mode="sem-wr-imm", update_value=0)
                            on_wait = (list(si.on_wait)
                                       if (k == 0 and si is not None and si.on_wait)
                                       else [])
                            ev.sync_info = mybir.SyncInfo(
                                on_wait=on_wait, on_update=[upd])
                            newlist.append(ev)
                        if si is not None and si.on_update:
                            evf = mybir.InstEventSemaphore(
                                name=f"{ins.name}-clrf", engine=ins.engine,
                                ins=[], outs=[])
                            evf.sync_info = mybir.SyncInfo(
                                on_wait=[], on_update=list(si.on_update))
                            newlist.append(evf)
                    changed = True
                    continue
                si = ins.sync_info
                if (si is not None and si.on_wait is not None
                        and len(si.on_wait) > 1):
                    waits = list(si.on_wait)
                    for i, w in enumerate(waits[1:]):
                        ev = mybir.InstEventSemaphore(
                            name=f"{ins.name}-hw{i}", engine=ins.engine,
                            ins=[], outs=[])
                        ev.sync_info = mybir.SyncInfo(on_wait=[w], on_update=[])
                        newlist.append(ev)
                    si.on_wait = [waits[0]]
                    ins.sync_info = si
                    changed = True
                newlist.append(ins)
            if changed:
                bb.instructions = newlist
    return nc


def build_bass(n_tok_per_batch, n_cores=NCORES):
    N = n_tok_per_batch
    T = B * N
    NT = T // 128             # token tiles
    KB = N // 128             # key tiles per batch
    QC = N // QW              # q chunks per batch
    OTC = max(1, T // 512)    # out-proj token chunks
    OTW = min(512, T)

    nc = bass.Bass(trn_type="TRN2", num_devices=n_cores)

    x = nc.dram_tensor("x", [T, C], F32, kind="ExternalInput")
    w_all = nc.dram_tensor("w_all", [C, QKV], BF16, kind="ExternalInput")
    b_all = nc.dram_tensor("b_all", [1, QKV], F32, kind="ExternalInput")
    gbe = nc.dram_tensor("gbe", [128, 4], F32, kind="ExternalInput")
    w_o_loc = nc.dram_tensor("w_o_loc", [CL, C], BF16, kind="ExternalInput")
    out_t = nc.dram_tensor("out_t", [C, T], F32, kind="ExternalOutput")

    with tile.TileContext(nc) as tc:
        _body(tc, x, w_all, b_all, gbe, w_o_loc, out_t,
              N=N, T=T, NT=NT, KB=KB, QC=QC, OTC=OTC, OTW=OTW,
              n_cores=n_cores)
    return _fixup_module(nc)


def _body(tc, x, w_all, b_all, gbe, w_o_loc, out_t,
          N, T, NT, KB, QC, OTC, OTW, n_cores):
    nc = tc.nc

    from contextlib import ExitStack
    octx = ExitStack()
    persist = octx.enter_context(tc.tile_pool(name="persist", bufs=1))

    ident = persist.tile([128, 128], BF16)
    make_identity(nc, ident)

    w_all_sb = persist.tile([128, KT, QKV], BF16)
    for kt in range(KT):
        nc.sync.dma_start(out=w_all_sb[:, kt, :],
                          in_=w_all[kt * 128:(kt + 1) * 128, :])
    b_sb = persist.tile([128, QKV], F32)
    nc.sync.dma_start(out=b_sb, in_=b_all.ap().to_broadcast([128, QKV]))
    gbe_sb = persist.tile([128, 4], F32)
    nc.sync.dma_start(out=gbe_sb, in_=gbe[:, :])
    w_o_sb = persist.tile([128, C], BF16)
    nc.sync.dma_start(out=w_o_sb, in_=w_o_loc[:, :])

    qT = persist.tile([128, T], BF16)      # [local col, token]
    kTt = persist.tile([128, T], BF16)
    v_aug = persist.tile([128, NT, 130], BF16)  # [tok%128, tile, head-block]
    q_pre = persist.tile([128, NT, 128], BF16)  # [tok%128, tile, local col]
    k_pre = persist.tile([128, NT, 128], BF16)
    stats = persist.tile([128, 4 * NT], F32)
    stats_all = persist.tile([128, 4 * NT], F32)
    o_un = persist.tile([128, T], BF16)    # unnormalized attention out^T
    rbc = persist.tile([128, T], BF16)     # 1/denominator broadcast map
    siluo = persist.tile([128, T], BF16)

    ones_col = persist.tile([128, NT], BF16)
    nc.vector.memset(ones_col, 1.0)
    nc.vector.tensor_copy(out=v_aug[:, :, 64:65], in_=ones_col)
    nc.vector.tensor_copy(out=v_aug[:, :, 129:130], in_=ones_col)

    # ---------------- phase 1: x-side LN+silu, transpose, QKV ----------------
    GB = 4  # token tiles per group
    with tc.tile_pool(name="ph1", bufs=2) as ph1, \
         tc.tile_pool(name="ph1x", bufs=2) as ph1x, \
         tc.tile_pool(name="ph1s", bufs=4) as ph1s, \
         tc.tile_pool(name="ph1p", bufs=2, space="PSUM") as ph1p, \
         tc.tile_pool(name="ph1q", bufs=3, space="PSUM") as ph1q:
        for g in range(NT // GB):
            xg = ph1x.tile([128, GB, C], F32, name="xg")
            nc.sync.dma_start(
                out=xg,
                in_=x[g * GB * 128:(g + 1) * GB * 128, :].rearrange(
                    "(t p) c -> p t c", p=128))

            stats6 = ph1s.tile([128, GB, 2, 6], F32, name="stats6")
            for t in range(GB):
                for h2 in range(2):
                    nc.vector.bn_stats(out=stats6[:, t, h2, :],
                                       in_=xg[:, t, h2 * 512:(h2 + 1) * 512])
            mv = ph1s.tile([128, GB, 2], F32, name="mv")
            for t in range(GB):
                nc.vector.bn_aggr(out=mv[:, t, :], in_=stats6[:, t, :, :])

            vpe = ph1s.tile([128, GB, 1], F32, name="vpe")
            nc.vector.tensor_scalar(out=vpe, in0=mv[:, :, 1:2], scalar1=EPS,
                                    scalar2=None, op0=ALU.add)
            rstd = _quake_rsqrt2(nc, ph1s, vpe, (128, GB, 1))
            nmr = ph1s.tile([128, GB, 1], F32, name="nmr")
            nc.vector.tensor_tensor(out=nmr, in0=mv[:, :, 0:1], in1=rstd,
                                    op=ALU.mult)
            nc.vector.tensor_scalar(out=nmr, in0=nmr, scalar1=-1.0,
                                    scalar2=None, op0=ALU.mult)

            # silu(LN(x)) -> bf16
            xs = ph1.tile([128, GB, C], BF16, name="xs")
            for t in range(GB):
                nc.scalar.activation(out=xs[:, t, :], in_=xg[:, t, :],
                                     func=AF.Silu,
                                     bias=nmr[:, t, :],
                                     scale=rstd[:, t, :])

            for t in range(GB):
                tt = g * GB + t
                pxT = ph1p.tile([128, 1024], BF16, name="pxT")
                for j in range(KT):
                    nc.tensor.transpose(pxT[:, j * 128:(j + 1) * 128],
                                        xs[:, t, j * 128:(j + 1) * 128],
                                        ident)
                xsT = ph1.tile([128, 1024], BF16, name="xsT")
                if t % 2 == 0:
                    nc.vector.tensor_copy(out=xsT, in_=pxT)
                else:
                    nc.scalar.copy(out=xsT, in_=pxT)

                pqkv = ph1q.tile([128, 512], F32, name="pqkv")
                for kt in range(KT):
                    nc.tensor.matmul(
                        pqkv[:, 0:QKV],
                        lhsT=xsT[:, kt * 128:(kt + 1) * 128],
                        rhs=w_all_sb[:, kt, :],
                        start=(kt == 0), stop=(kt == KT - 1))

                nc.vector.scalar_tensor_tensor(
                    out=q_pre[:, tt, :], in0=pqkv[:, 0:128], scalar=1.0,
                    in1=b_sb[:, 0:128], op0=ALU.mult, op1=ALU.add)
                nc.vector.scalar_tensor_tensor(
                    out=k_pre[:, tt, :], in0=pqkv[:, 128:256], scalar=1.0,
                    in1=b_sb[:, 128:256], op0=ALU.mult, op1=ALU.add)
                nc.vector.scalar_tensor_tensor(
                    out=v_aug[:, tt, :].rearrange("p (h e) -> p h e", e=65)[:, :, 0:64],
                    in0=pqkv[:, 256:384].rearrange("p (h e) -> p h e", e=64),
                    scalar=1.0,
                    in1=b_sb[:, 256:384].rearrange("p (h e) -> p h e", e=64),
                    op0=ALU.mult, op1=ALU.add)

    # ---------------- phase 2: q/k stats + AllReduce ----------------
    with tc.tile_pool(name="ph2", bufs=1) as ph2:
        nc.vector.tensor_reduce(out=stats[:, 0:NT], in_=q_pre[:, :, :],
                                axis=AX.X, op=ALU.add)
        nc.vector.tensor_reduce(out=stats[:, 2 * NT:3 * NT], in_=k_pre[:, :, :],
                                axis=AX.X, op=ALU.add)
        scr = ph2.tile([128, 128], BF16, name="scr")
        for tt in range(NT):
            nc.scalar.activation(
                out=scr, in_=q_pre[:, tt, :], func=AF.Square,
                accum_out=stats[:, NT + tt:NT + tt + 1])
        for tt in range(NT):
            nc.scalar.activation(
                out=scr, in_=k_pre[:, tt, :], func=AF.Square,
                accum_out=stats[:, 3 * NT + tt:3 * NT + tt + 1])

        with tc.tile_pool(name="dram", bufs=1, space="DRAM") as dpool:
            cc_in = dpool.tile([128, 4 * NT], F32, name="cc_in")
            cc_out = dpool.tile([128, 4 * NT], F32, name="cc_out",
                                addr_space="Shared")
            nc.sync.dma_start(out=cc_in, in_=stats)
            nc.gpsimd.collective_compute(
                "AllReduce", ALU.add,
                replica_groups=[list(range(n_cores))],
                ins=[cc_in.opt()], outs=[cc_out.opt()])
            nc.sync.dma_start(out=stats_all, in_=cc_out)

        # per-token mean / rstd for q and k (over full 1024-wide inner dim)
        qk_stats = []
        for which in range(2):  # 0 -> q, 1 -> k
            s_sum = stats_all[:, 2 * which * NT:(2 * which + 1) * NT]
            s_ssq = stats_all[:, (2 * which + 1) * NT:(2 * which + 2) * NT]
            m = ph2.tile([128, NT], F32, name=f"m_{which}")
            nc.vector.tensor_scalar(out=m, in0=s_sum, scalar1=1.0 / INNER,
                                    scalar2=None, op0=ALU.mult)
            msq = ph2.tile([128, NT], F32, name=f"msq_{which}")
            nc.vector.tensor_scalar(out=msq, in0=s_ssq, scalar1=1.0 / INNER,
                                    scalar2=None, op0=ALU.mult)
            tmp = ph2.tile([128, NT], F32, name=f"tmp_{which}")
            nc.vector.tensor_tensor(out=tmp, in0=m, in1=m, op=ALU.mult)
            nc.vector.tensor_tensor(out=tmp, in0=msq, in1=tmp, op=ALU.subtract)
            nc.vector.tensor_scalar(out=tmp, in0=tmp, scalar1=EPS,
                                    scalar2=None, op0=ALU.add)
            rstd = _quake_rsqrt(nc, ph2, tmp, (128, NT), suffix=f"_{which}")
            nmr = ph2.tile([128, NT], F32, name=f"nmr_{which}")
            nc.vector.tensor_tensor(out=nmr, in0=m, in1=rstd, op=ALU.mult)
            nc.vector.tensor_scalar(out=nmr, in0=nmr, scalar1=-1.0,
                                    scalar2=None, op0=ALU.mult)
            qk_stats.append((m, rstd, nmr))

        # ---------------- phase 3: apply LN, transpose q/k ----------------
        with tc.tile_pool(name="ph3", bufs=8) as ph3, \
             tc.tile_pool(name="ph3p", bufs=4, space="PSUM") as ph3p:
            for which, (pre, dst, gcol) in enumerate(
                    [(q_pre, qT, 0), (k_pre, kTt, 2)]):
                m, rstd, nmr = qk_stats[which]
                for tt in range(NT):
                    qn = ph3.tile([128, 128], BF16, name="qn")
                    if which == 1:
                        # k: normalize on ACT so DVE and ACT each carry one
                        # of the two per-tile passes
                        nc.scalar.activation(
                            out=qn, in_=pre[:, tt, :], func=AF.Identity,
                            bias=nmr[:, tt:tt + 1],
                            scale=rstd[:, tt:tt + 1])
                    else:
                        nc.vector.tensor_scalar(
                            out=qn, in0=pre[:, tt, :],
                            scalar1=m[:, tt:tt + 1],
                            scalar2=rstd[:, tt:tt + 1],
                            op0=ALU.subtract, op1=ALU.mult)
                    pq = ph3p.tile([128, 128], BF16, name="pq")
                    nc.tensor.transpose(pq, qn, ident)
                    if which == 0:
                        nc.scalar.activation(
                            out=dst[:, tt * 128:(tt + 1) * 128], in_=pq,
                            func=AF.Identity,
                            bias=gbe_sb[:, gcol + 1:gcol + 2],
                            scale=gbe_sb[:, gcol:gcol + 1])
                    else:
                        nc.vector.tensor_scalar(
                            out=dst[:, tt * 128:(tt + 1) * 128], in0=pq,
                            scalar1=gbe_sb[:, gcol:gcol + 1],
                            scalar2=gbe_sb[:, gcol + 1:gcol + 2],
                            op0=ALU.mult, op1=ALU.add)

    # ---------------- phase 4: attention, heads interleaved ----------------
    with tc.tile_pool(name="att", bufs=3) as att, \
         tc.tile_pool(name="attd", bufs=2) as attd, \
         tc.tile_pool(name="dramd", bufs=1, space="DRAM") as dramd, \
         tc.tile_pool(name="attp", bufs=2, space="PSUM") as attp, \
         tc.tile_pool(name="attpo", bufs=2, space="PSUM") as attpo:
        d_dram = dramd.tile([1, B * HL * N], F32, name="d_dram")
        r_dram = dramd.tile([1, B * HL * N], BF16, name="r_dram")
        for b in range(B):
            for qc in range(QC):
                q0 = b * N + qc * QW
                pO = [attpo.tile([128, QW], F32, name=f"pO{h}")
                      for h in range(HL)]
                for kb in range(KB):
                    vt = b * KB + kb
                    pS = [attp.tile([128, QW], F32, name=f"pS{h}")
                          for h in range(HL)]
                    for h in range(HL):
                        # K=64 with base partition h*64: the two heads run
                        # concurrently on disjoint PE row-groups
                        nc.tensor.matmul(
                            pS[h],
                            lhsT=kTt[h * 64:(h + 1) * 64,
                                     b * N + kb * 128:b * N + (kb + 1) * 128],
                            rhs=qT[h * 64:(h + 1) * 64, q0:q0 + QW],
                            start=True, stop=True)
                    eS = []
                    for h in range(HL):
                        e = att.tile([128, QW], BF16, name=f"eS{h}")
                        nc.scalar.activation(out=e, in_=pS[h], func=AF.Exp)
                        eS.append(e)
                    for h in range(HL):
                        nc.tensor.matmul(
                            pO[h][0:65, :],
                            lhsT=v_aug[:, vt, h * 65:(h + 1) * 65],
                            rhs=eS[h],
                            start=(kb == 0), stop=(kb == KB - 1))

                # evict unnormalized O and denominators
                for h in range(HL):
                    doff = (b * HL + h) * N + qc * QW
                    dsl = attd.tile([1, QW], F32, name=f"dsl{h}")
                    if h == 0:
                        nc.vector.tensor_copy(
                            out=o_un[h * 64:(h + 1) * 64, q0:q0 + QW],
                            in_=pO[h][0:64, :])
                        nc.scalar.copy(out=dsl, in_=pO[h][64:65, :])
                    else:
                        nc.scalar.copy(
                            out=o_un[h * 64:(h + 1) * 64, q0:q0 + QW],
                            in_=pO[h][0:64, :])
                        nc.vector.tensor_copy(out=dsl, in_=pO[h][64:65, :])
                    nc.sync.dma_start(out=d_dram[:, doff:doff + QW], in_=dsl)

        # batched reciprocal of all B*HL*N denominators across 128 partitions
        with tc.tile_pool(name="attr", bufs=1) as attr:
            nd = B * HL * N
            dg = attr.tile([128, nd // 128], F32, name="dg")
            nc.sync.dma_start(
                out=dg, in_=d_dram[0, :].rearrange("(p f) -> p f", p=128))
            rg = attr.tile([128, nd // 128], F32, name="rg")
            nc.vector.reciprocal(out=rg, in_=dg)
            rgb = attr.tile([128, nd // 128], BF16, name="rgb")
            nc.vector.tensor_copy(out=rgb, in_=rg)
            nc.sync.dma_start(
                out=r_dram[0, :].rearrange("(p f) -> p f", p=128), in_=rgb)
            # broadcast back: rows h*64..h*64+63 get head h's recip map
            for h in range(HL):
                nc.sync.dma_start(
                    out=rbc[h * 64:(h + 1) * 64, :].rearrange(
                        "p (b t) -> p b t", b=B),
                    in_=r_dram[:, :].rearrange(
                        "o (b hh t) -> o b hh t", b=B, hh=HL)[:, :, h, :]
                        .to_broadcast([64, B, N]))

        # normalize + silu
        onrm = att.tile([128, T], BF16, name="onrm")
        for ch in range(T // 1024):
            nc.vector.tensor_tensor(
                out=onrm[:, ch * 1024:(ch + 1) * 1024],
                in0=o_un[:, ch * 1024:(ch + 1) * 1024],
                in1=rbc[:, ch * 1024:(ch + 1) * 1024], op=ALU.mult)
            nc.scalar.activation(
                out=siluo[:, ch * 1024:(ch + 1) * 1024],
                in_=onrm[:, ch * 1024:(ch + 1) * 1024], func=AF.Silu)

    # ---------------- phase 5: output projection ----------------
    with tc.tile_pool(name="ph5", bufs=4) as ph5, \
         tc.tile_pool(name="ph5p", bufs=4, space="PSUM") as ph5p:
        for ct in range(KT):
            for tk in range(OTC):
                po = ph5p.tile([128, OTW], F32, name="po")
                nc.tensor.matmul(
                    po,
                    lhsT=w_o_sb[:, ct * 128:(ct + 1) * 128],
                    rhs=siluo[:, tk * OTW:(tk + 1) * OTW],
                    start=True, stop=True)
                ev = ph5.tile([128, OTW], F32, name="ev")
                if (ct * OTC + tk) % 2 == 0:
                    nc.vector.tensor_copy(out=ev, in_=po)
                else:
                    nc.scalar.copy(out=ev, in_=po)
                nc.sync.dma_start(
                    out=out_t[ct * 128:(ct + 1) * 128,
                              tk * OTW:(tk + 1) * OTW],
                    in_=ev)

    octx.close()


def make_in_maps(inputs, n_tok_per_batch, n_cores=NCORES):
    """Slice full inputs into per-core input maps (head sharding)."""
    import ml_dtypes
    bf16 = ml_dtypes.bfloat16
    x = np.ascontiguousarray(np.asarray(inputs["x"], np.float32)
                             .reshape(B * n_tok_per_batch, C))
    w_q = np.asarray(inputs["w_q"], np.float32)
    w_k = np.asarray(inputs["w_k"], np.float32)
    w_v = np.asarray(inputs["w_v"], np.float32)
    b_q = np.asarray(inputs["b_q"], np.float32)
    b_k = np.asarray(inputs["b_k"], np.float32)
    b_v = np.asarray(inputs["b_v"], np.float32)
    g_q = np.asarray(inputs["g_q"], np.float32)
    be_q = np.asarray(inputs["be_q"], np.float32)
    g_k = np.asarray(inputs["g_k"], np.float32)
    be_k = np.asarray(inputs["be_k"], np.float32)
    w_o = np.asarray(inputs["w_o"], np.float32)

    scale = float(INNER) ** -0.5
    in_maps = []
    for c in range(n_cores):
        cols = slice(c * CL, (c + 1) * CL)
        w_all = np.ascontiguousarray(
            np.concatenate([w_q[:, cols], w_k[:, cols], w_v[:, cols]],
                           axis=1)).astype(bf16)
        b_all = np.ascontiguousarray(
            np.concatenate([b_q[cols], b_k[cols], b_v[cols]])[None, :])
        gbe = np.ascontiguousarray(
            np.stack([g_q[cols] * scale, be_q[cols] * scale,
                      g_k[cols], be_k[cols]], axis=1))
        w_o_c = np.ascontiguousarray(w_o[cols, :]).astype(bf16)
        in_maps.append({
            "x": x, "w_all": w_all, "b_all": b_all,
            "gbe": gbe, "w_o_loc": w_o_c,
        })
    return in_maps


def combine_outputs(out_ts, inputs, n_tok_per_batch):
    b_o = np.asarray(inputs["b_o"], np.float32)
    acc = np.zeros_like(out_ts[0], dtype=np.float64)
    for o in out_ts:
        acc += o.astype(np.float64)
    out = acc.T.astype(np.float32) + b_o[None, :]
    return out.reshape(B, n_tok_per_batch, C).astype(np.float32)


_NC_CACHE = {}


def kernel(**inputs):
    from concourse.bass_utils import run_bass_kernel_spmd

    n_tok = np.asarray(inputs["x"]).shape[1]
    if n_tok not in _NC_CACHE:
        _NC_CACHE[n_tok] = build_bass(n_tok)
    nc = _NC_CACHE[n_tok]
    in_maps = make_in_maps(inputs, n_tok)
    res = run_bass_kernel_spmd(nc, in_maps, core_ids=list(range(NCORES)))
    out_ts = [r["out_t"] for r in res.results]
    return combine_outputs(out_ts, inputs, n_tok)


# revision 8
# speedup vs baseline: 1.2858x; 1.1865x over previous
"""Trainium2 Bass kernel for nn_Attention_71846212928150.

Self-attention block (pre-LN + silu, QKV projections, per-head attention with
q/k LayerNorms, output projection), sharded over 8 NeuronCores by heads:
core c owns heads {2c, 2c+1} = inner columns [128c, 128c+128).

v2: bf16 matmul datapath + head-interleaved attention.
  phase 1: stream x in 128-token tiles; LN (bn_stats + quake-rsqrt on DVE)
           fused with silu on ACT emitting bf16; PE-transpose (bf16);
           fused QKV matmul (bf16) into [tok, 384] PSUM; evict q/k/v bf16.
  phase 2: partial sum / sumsq of q,k over the local 128 columns;
           AllReduce [128,128] f32 stats across the 8 cores.
  phase 3: apply q/k LayerNorm in [tok, col] layout, PE-transpose to
           [col, tok] bf16, apply gain/bias (inner**-0.5 folded into q gain).
  phase 4: per (batch, q-chunk): both heads interleaved. S matmuls have K=64
           and per-head base partitions 0/64, so the two heads' S matmuls run
           CONCURRENTLY on disjoint PE row-groups (auto tile_position).
           exp on ACT (no max subtraction; |scores| small by construction)
           emits bf16; PV matmul with ones-column in V accumulates the
           softmax denominator in PSUM row 64.
  phase 4b: batched denominator reciprocal: gather all [1,512] denominator
           rows to DRAM, reload as [128,64], one DVE reciprocal, scatter-
           broadcast back to [128, T] via DMA; normalize + silu in two
           full-width passes.
  phase 5: output projection (bf16) -> out^T [1024, 4096] f32 partial sums,
           host adds the 8 partials, transposes, adds b_o.
"""

import numpy as np

import concourse.bass as bass
import concourse.mybir as mybir
import concourse.tile as tile
from concourse.masks import make_identity

F32 = mybir.dt.float32
BF16 = mybir.dt.bfloat16
I32 = mybir.dt.int32
AF = mybir.ActivationFunctionType
ALU = mybir.AluOpType
AX = mybir.AxisListType

B = 2
C = 1024
H = 16
DH = 64
INNER = H * DH
NCORES = 8
HL = H // NCORES          # 2 heads per core
CL = HL * DH              # 128 local inner columns
QKV = 3 * CL              # 384
KT = C // 128             # 8 contraction tiles over C
EPS = 1e-5
MAGIC = 0x5F3759DF
QW = 512                  # attention q-chunk width


def _quake_rsqrt(nc, pool, vpe, shape, suffix=""):
    """rstd = 1/sqrt(vpe) entirely on DVE (fp32-exact after 3 Newton steps)."""
    y = pool.tile(list(shape), F32, name=f"qk_y{suffix}")
    t2 = pool.tile(list(shape), F32, name=f"qk_t2{suffix}")
    nc.vector.tensor_scalar(
        out=y.bitcast(I32), in0=vpe.bitcast(I32), scalar1=1, scalar2=None,
        op0=ALU.logical_shift_right)
    nc.vector.tensor_scalar(
        out=y.bitcast(I32), in0=y.bitcast(I32), scalar1=-1, scalar2=MAGIC,
        op0=ALU.mult, op1=ALU.add)
    for _ in range(3):
        nc.vector.tensor_tensor(out=t2, in0=y, in1=y, op=ALU.mult)
        nc.vector.tensor_tensor(out=t2, in0=t2, in1=vpe, op=ALU.mult)
        nc.vector.tensor_scalar(out=t2, in0=t2, scalar1=-0.5, scalar2=1.5,
                                op0=ALU.mult, op1=ALU.add)
        nc.vector.tensor_tensor(out=y, in0=y, in1=t2, op=ALU.mult)
    return y


def _quake_rsqrt2(nc, pool, vpe, shape, suffix=""):
    """Two-iteration variant (~4e-6 rel err) for the latency-critical x path."""
    y = pool.tile(list(shape), F32, name=f"qj_y{suffix}")
    t2 = pool.tile(list(shape), F32, name=f"qj_t2{suffix}")
    nc.vector.tensor_scalar(
        out=y.bitcast(I32), in0=vpe.bitcast(I32), scalar1=1, scalar2=None,
        op0=ALU.logical_shift_right)
    nc.vector.tensor_scalar(
        out=y.bitcast(I32), in0=y.bitcast(I32), scalar1=-1, scalar2=MAGIC,
        op0=ALU.mult, op1=ALU.add)
    for _ in range(2):
        nc.vector.tensor_tensor(out=t2, in0=y, in1=y, op=ALU.mult)
        nc.vector.tensor_tensor(out=t2, in0=t2, in1=vpe, op=ALU.mult)
        nc.vector.tensor_scalar(out=t2, in0=t2, scalar1=-0.5, scalar2=1.5,
                                op0=ALU.mult, op1=ALU.add)
        nc.vector.tensor_tensor(out=y, in0=y, in1=t2, op=ALU.mult)
    return y


def _fixup_module(nc):
    """Adapt Tile-emitted BIR to this container's walrus build.

    1. The tail `EVENT_SEMAPHORE_RANGE_CLEAR` InstISA (opcode 176) is not
       understood by this walrus' birverifier. Replace it with one
       EventSemaphore sem-write-0 per semaphore in the cleared range
       (functionally equivalent, re-execution stays safe).
    2. Drain instructions carrying more than one semaphore wait fail codegen
       ("Too many sync wait commands"). Hoist the extra waits into standalone
       EventSemaphore wait instructions just before the drain.
    """
    for f in nc.m.functions:
        for bb in f.blocks:
            newlist = []
            changed = False
            for ins in bb.instructions:
                tn = type(ins).__name__
                if tn == "InstISA" and getattr(ins, "isa_opcode", None) == 176:
                    ad = ins.ant_dict or {}
                    first = ad.get("range_first")
                    last = ad.get("range_last")
                    if first is not None and last is not None:
                        si = ins.sync_info
                        sems = list(range(first, last + 1))
                        for k, sem in enumerate(sems):
                            ev = mybir.InstEventSemaphore(
                                name=f"{ins.name}-clr{k}", engine=ins.engine,
                                ins=[], outs=[])
                            upd = mybir.SyncUpdate(
                                sync_type="semaphore", id=sem,
                                update_mode="sem-wr-imm", update_value=0)
                            on_wait = (list(si.on_wait)
                                       if (k == 0 and si is not None and si.on_wait)
                                       else [])
                            ev.sync_info = mybir.SyncInfo(
                                on_wait=on_wait, on_update=[upd])
                            newlist.append(ev)
                        if si is not None and si.on_update:
                            evf = mybir.InstEventSemaphore(
                                name=f"{ins.name}-clrf", engine=ins.engine,
                                ins=[], outs=[])
                            evf.sync_info = mybir.SyncInfo(
                                on_wait=[], on_update=list(si.on_update))
                            newlist.append(evf)
                    changed = True
                    continue
                si = ins.sync_info
                if (si is not None and si.on_wait is not None
                        and len(si.on_wait) > 1):
                    waits = list(si.on_wait)
                    for i, w in enumerate(waits[1:]):
                        ev = mybir.InstEventSemaphore(
                            name=f"{ins.name}-hw{i}", engine=ins.engine,
                            ins=[], outs=[])
                        ev.sync_info = mybir.SyncInfo(on_wait=[w], on_update=[])
                        newlist.append(ev)
                    si.on_wait = [waits[0]]
                    ins.sync_info = si
                    changed = True
                newlist.append(ins)
            if changed:
                bb.instructions = newlist
    return nc


def build_bass(n_tok_per_batch, n_cores=NCORES):
    N = n_tok_per_batch
    T = B * N
    NT = T // 128             # token tiles
    KB = N // 128             # key tiles per batch
    QC = N // QW              # q chunks per batch
    OTC = max(1, T // 512)    # out-proj token chunks
    OTW = min(512, T)

    nc = bass.Bass(trn_type="TRN2", num_devices=n_cores)

    x = nc.dram_tensor("x", [T, C], F32, kind="ExternalInput")
    w_all = nc.dram_tensor("w_all", [C, QKV], BF16, kind="ExternalInput")
    b_all = nc.dram_tensor("b_all", [1, QKV], F32, kind="ExternalInput")
    gbe = nc.dram_tensor("gbe", [128, 4], F32, kind="ExternalInput")
    w_o_loc = nc.dram_tensor("w_o_loc", [CL, C], BF16, kind="ExternalInput")
    out_t = nc.dram_tensor("out_t", [C, T], F32, kind="ExternalOutput")

    with tile.TileContext(nc) as tc:
        _body(tc, x, w_all, b_all, gbe, w_o_loc, out_t,
              N=N, T=T, NT=NT, KB=KB, QC=QC, OTC=OTC, OTW=OTW,
              n_cores=n_cores)
    return _fixup_module(nc)


def _body(tc, x, w_all, b_all, gbe, w_o_loc, out_t,
          N, T, NT, KB, QC, OTC, OTW, n_cores):
    nc = tc.nc

    from contextlib import ExitStack
    octx = ExitStack()
    persist = octx.enter_context(tc.tile_pool(name="persist", bufs=1))

    ident = persist.tile([128, 128], BF16)
    make_identity(nc, ident)

    w_all_sb = persist.tile([128, KT, QKV], BF16)
    for kt in range(KT):
        nc.sync.dma_start(out=w_all_sb[:, kt, :],
                          in_=w_all[kt * 128:(kt + 1) * 128, :])
    b_sb = persist.tile([128, QKV], F32)
    nc.sync.dma_start(out=b_sb, in_=b_all.ap().to_broadcast([128, QKV]))
    gbe_sb = persist.tile([128, 4], F32)
    nc.sync.dma_start(out=gbe_sb, in_=gbe[:, :])
    w_o_sb = persist.tile([128, C], BF16)
    nc.sync.dma_start(out=w_o_sb, in_=w_o_loc[:, :])

    qT = persist.tile([128, T], BF16)      # [local col, token]
    kTt = persist.tile([128, T], BF16)
    v_aug = persist.tile([128, NT, 130], BF16)  # [tok%128, tile, head-block]
    q_pre = persist.tile([128, NT, 128], BF16)  # [tok%128, tile, local col]
    k_pre = persist.tile([128, NT, 128], BF16)
    stats = persist.tile([128, 4 * NT], F32)
    stats_all = persist.tile([128, 4 * NT], F32)
    onrm = persist.tile([128, T], BF16)    # normalized attention out^T
    siluo = persist.tile([128, T], BF16)

    ones_col = persist.tile([128, NT], BF16)
    nc.vector.memset(ones_col, 1.0)
    nc.vector.tensor_copy(out=v_aug[:, :, 64:65], in_=ones_col)
    nc.vector.tensor_copy(out=v_aug[:, :, 129:130], in_=ones_col)

    # ---------------- phase 1: x-side LN+silu, transpose, QKV ----------------
    GB = 4  # token tiles per group
    with tc.tile_pool(name="ph1", bufs=2) as ph1, \
         tc.tile_pool(name="ph1x", bufs=2) as ph1x, \
         tc.tile_pool(name="ph1s", bufs=4) as ph1s, \
         tc.tile_pool(name="ph1p", bufs=2, space="PSUM") as ph1p, \
         tc.tile_pool(name="ph1q", bufs=3, space="PSUM") as ph1q:
        for g in range(NT // GB):
            xg = ph1x.tile([128, GB, C], F32, name="xg")
            nc.sync.dma_start(
                out=xg,
                in_=x[g * GB * 128:(g + 1) * GB * 128, :].rearrange(
                    "(t p) c -> p t c", p=128))

            stats6 = ph1s.tile([128, GB, 2, 6], F32, name="stats6")
            for t in range(GB):
                for h2 in range(2):
                    nc.vector.bn_stats(out=stats6[:, t, h2, :],
                                       in_=xg[:, t, h2 * 512:(h2 + 1) * 512])
            mv = ph1s.tile([128, GB, 2], F32, name="mv")
            for t in range(GB):
                nc.vector.bn_aggr(out=mv[:, t, :], in_=stats6[:, t, :, :])

            vpe = ph1s.tile([128, GB, 1], F32, name="vpe")
            nc.vector.tensor_scalar(out=vpe, in0=mv[:, :, 1:2], scalar1=EPS,
                                    scalar2=None, op0=ALU.add)
            rstd = _quake_rsqrt2(nc, ph1s, vpe, (128, GB, 1))
            nmr = ph1s.tile([128, GB, 1], F32, name="nmr")
            nc.vector.tensor_tensor(out=nmr, in0=mv[:, :, 0:1], in1=rstd,
                                    op=ALU.mult)
            nc.vector.tensor_scalar(out=nmr, in0=nmr, scalar1=-1.0,
                                    scalar2=None, op0=ALU.mult)

            # silu(LN(x)) -> bf16
            xs = ph1.tile([128, GB, C], BF16, name="xs")
            for t in range(GB):
                nc.scalar.activation(out=xs[:, t, :], in_=xg[:, t, :],
                                     func=AF.Silu,
                                     bias=nmr[:, t, :],
                                     scale=rstd[:, t, :])

            for t in range(GB):
                tt = g * GB + t
                pxT = ph1p.tile([128, 1024], BF16, name="pxT")
                for j in range(KT):
                    nc.tensor.transpose(pxT[:, j * 128:(j + 1) * 128],
                                        xs[:, t, j * 128:(j + 1) * 128],
                                        ident)
                xsT = ph1.tile([128, 1024], BF16, name="xsT")
                if t % 2 == 0:
                    nc.vector.tensor_copy(out=xsT, in_=pxT)
                else:
                    nc.scalar.copy(out=xsT, in_=pxT)

                pqkv = ph1q.tile([128, 512], F32, name="pqkv")
                for kt in range(KT):
                    nc.tensor.matmul(
                        pqkv[:, 0:QKV],
                        lhsT=xsT[:, kt * 128:(kt + 1) * 128],
                        rhs=w_all_sb[:, kt, :],
                        start=(kt == 0), stop=(kt == KT - 1))

                nc.vector.scalar_tensor_tensor(
                    out=q_pre[:, tt, :], in0=pqkv[:, 0:128], scalar=1.0,
                    in1=b_sb[:, 0:128], op0=ALU.mult, op1=ALU.add)
                nc.vector.scalar_tensor_tensor(
                    out=k_pre[:, tt, :], in0=pqkv[:, 128:256], scalar=1.0,
                    in1=b_sb[:, 128:256], op0=ALU.mult, op1=ALU.add)
                nc.vector.scalar_tensor_tensor(
                    out=v_aug[:, tt, :].rearrange("p (h e) -> p h e", e=65)[:, :, 0:64],
                    in0=pqkv[:, 256:384].rearrange("p (h e) -> p h e", e=64),
                    scalar=1.0,
                    in1=b_sb[:, 256:384].rearrange("p (h e) -> p h e", e=64),
                    op0=ALU.mult, op1=ALU.add)

    # ---------------- phase 2: q/k stats + AllReduce ----------------
    with tc.tile_pool(name="ph2", bufs=1) as ph2:
        nc.vector.tensor_reduce(out=stats[:, 0:NT], in_=q_pre[:, :, :],
                                axis=AX.X, op=ALU.add)
        nc.vector.tensor_reduce(out=stats[:, 2 * NT:3 * NT], in_=k_pre[:, :, :],
                                axis=AX.X, op=ALU.add)
        scr = ph2.tile([128, 128], BF16, name="scr")
        for tt in range(NT):
            nc.scalar.activation(
                out=scr, in_=q_pre[:, tt, :], func=AF.Square,
                accum_out=stats[:, NT + tt:NT + tt + 1])
        for tt in range(NT):
            nc.scalar.activation(
                out=scr, in_=k_pre[:, tt, :], func=AF.Square,
                accum_out=stats[:, 3 * NT + tt:3 * NT + tt + 1])

        with tc.tile_pool(name="dram", bufs=1, space="DRAM") as dpool:
            cc_in = dpool.tile([128, 4 * NT], F32, name="cc_in")
            cc_out = dpool.tile([128, 4 * NT], F32, name="cc_out",
                                addr_space="Shared")
            nc.sync.dma_start(out=cc_in, in_=stats)
            nc.gpsimd.collective_compute(
                "AllReduce", ALU.add,
                replica_groups=[list(range(n_cores))],
                ins=[cc_in.opt()], outs=[cc_out.opt()])
            nc.sync.dma_start(out=stats_all, in_=cc_out)

        # per-token mean / rstd for q and k (over full 1024-wide inner dim)
        qk_stats = []
        for which in range(2):  # 0 -> q, 1 -> k
            s_sum = stats_all[:, 2 * which * NT:(2 * which + 1) * NT]
            s_ssq = stats_all[:, (2 * which + 1) * NT:(2 * which + 2) * NT]
            m = ph2.tile([128, NT], F32, name=f"m_{which}")
            nc.vector.tensor_scalar(out=m, in0=s_sum, scalar1=1.0 / INNER,
                                    scalar2=None, op0=ALU.mult)
            msq = ph2.tile([128, NT], F32, name=f"msq_{which}")
            nc.vector.tensor_scalar(out=msq, in0=s_ssq, scalar1=1.0 / INNER,
                                    scalar2=None, op0=ALU.mult)
            tmp = ph2.tile([128, NT], F32, name=f"tmp_{which}")
            nc.vector.tensor_tensor(out=tmp, in0=m, in1=m, op=ALU.mult)
            nc.vector.tensor_tensor(out=tmp, in0=msq, in1=tmp, op=ALU.subtract)
            nc.vector.tensor_scalar(out=tmp, in0=tmp, scalar1=EPS,
                                    scalar2=None, op0=ALU.add)
            rstd = _quake_rsqrt(nc, ph2, tmp, (128, NT), suffix=f"_{which}")
            nmr = ph2.tile([128, NT], F32, name=f"nmr_{which}")
            nc.vector.tensor_tensor(out=nmr, in0=m, in1=rstd, op=ALU.mult)
            nc.vector.tensor_scalar(out=nmr, in0=nmr, scalar1=-1.0,
                                    scalar2=None, op0=ALU.mult)
            qk_stats.append((m, rstd, nmr))

        # ---------------- phase 3: apply LN, transpose q/k ----------------
        with tc.tile_pool(name="ph3", bufs=8) as ph3, \
             tc.tile_pool(name="ph3p", bufs=4, space="PSUM") as ph3p:
            for which, (pre, dst, gcol) in enumerate(
                    [(q_pre, qT, 0), (k_pre, kTt, 2)]):
                m, rstd, nmr = qk_stats[which]
                for tt in range(NT):
                    qn = ph3.tile([128, 128], BF16, name="qn")
                    if which == 1:
                        # k: normalize on ACT so DVE and ACT each carry one
                        # of the two per-tile passes
                        nc.scalar.activation(
                            out=qn, in_=pre[:, tt, :], func=AF.Identity,
                            bias=nmr[:, tt:tt + 1],
                            scale=rstd[:, tt:tt + 1])
                    else:
                        nc.vector.tensor_scalar(
                            out=qn, in0=pre[:, tt, :],
                            scalar1=m[:, tt:tt + 1],
                            scalar2=rstd[:, tt:tt + 1],
                            op0=ALU.subtract, op1=ALU.mult)
                    pq = ph3p.tile([128, 128], BF16, name="pq")
                    nc.tensor.transpose(pq, qn, ident)
                    if which == 0:
                        nc.scalar.activation(
                            out=dst[:, tt * 128:(tt + 1) * 128], in_=pq,
                            func=AF.Identity,
                            bias=gbe_sb[:, gcol + 1:gcol + 2],
                            scale=gbe_sb[:, gcol:gcol + 1])
                    else:
                        nc.vector.tensor_scalar(
                            out=dst[:, tt * 128:(tt + 1) * 128], in0=pq,
                            scalar1=gbe_sb[:, gcol:gcol + 1],
                            scalar2=gbe_sb[:, gcol + 1:gcol + 2],
                            op0=ALU.mult, op1=ALU.add)

    # -------- phase 4+5: attention (heads interleaved) + out-proj --------
    # Schraudolph fast-exp emitting bf16 bits directly (DVE, one op):
    LOG2E = 1.4426950408889634
    A16 = 128.0 * LOG2E
    B16 = float(0x3F80) - 486408.0 / 65536.0
    I16 = mybir.dt.int16

    with tc.tile_pool(name="att", bufs=3) as att, \
         tc.tile_pool(name="attd", bufs=2) as attd, \
         tc.tile_pool(name="ph5", bufs=4) as ph5, \
         tc.tile_pool(name="dramd", bufs=2, space="DRAM") as dramd, \
         tc.tile_pool(name="attp", bufs=2, space="PSUM") as attp, \
         tc.tile_pool(name="attpo", bufs=1, space="PSUM") as attpo, \
         tc.tile_pool(name="ph5p", bufs=2, space="PSUM") as ph5p:

        def emit_out_chunk(ci):
            # out-projection for token chunk ci (reads siluo[:, ci*QW:...])
            for ct in range(KT):
                po = ph5p.tile([128, QW], F32, name="po")
                nc.tensor.matmul(
                    po,
                    lhsT=w_o_sb[:, ct * 128:(ct + 1) * 128],
                    rhs=siluo[:, ci * QW:(ci + 1) * QW],
                    start=True, stop=True)
                ev = ph5.tile([128, QW], F32, name="ev")
                if ct % 2 == 0:
                    nc.vector.tensor_copy(out=ev, in_=po)
                else:
                    nc.scalar.copy(out=ev, in_=po)
                nc.sync.dma_start(
                    out=out_t[ct * 128:(ct + 1) * 128,
                              ci * QW:(ci + 1) * QW],
                    in_=ev)

        def emit_norm_chunk(desc):
            # deferred: normalize + silu for a finished chunk (rb has landed)
            ci, q0, ous, rbs = desc
            for h in range(HL):
                nc.vector.tensor_tensor(
                    out=onrm[h * 64:(h + 1) * 64, q0:q0 + QW],
                    in0=ous[h], in1=rbs[h], op=ALU.mult)
            nc.scalar.activation(out=siluo[:, q0:q0 + QW],
                                 in_=onrm[:, q0:q0 + QW], func=AF.Silu)

        chunks = [(b, qc) for b in range(B) for qc in range(QC)]
        norm_pend = []
        for ci, (b, qc) in enumerate(chunks):
            q0 = b * N + qc * QW
            pO = [attpo.tile([128, QW], F32, name=f"pO{h}")
                  for h in range(HL)]
            pv_pend = []
            for kb in range(KB):
                pS = [attp.tile([128, QW], F32, name=f"pS{h}")
                      for h in range(HL)]
                for h in range(HL):
                    # K=64 with base partition h*64: the two heads' S matmuls
                    # are adjacent in the PE stream and run concurrently on
                    # disjoint row-groups (auto tile_position)
                    nc.tensor.matmul(
                        pS[h],
                        lhsT=kTt[h * 64:(h + 1) * 64,
                                 b * N + kb * 128:b * N + (kb + 1) * 128],
                        rhs=qT[h * 64:(h + 1) * 64, q0:q0 + QW],
                        start=True, stop=True)
                e0 = att.tile([128, QW], BF16, name="eS0")
                nc.scalar.activation(out=e0, in_=pS[0], func=AF.Exp)
                e1 = att.tile([128, QW], BF16, name="eS1")
                nc.vector.tensor_scalar(
                    out=e1.bitcast(I16), in0=pS[1], scalar1=A16, scalar2=B16,
                    op0=ALU.mult, op1=ALU.add)
                pv_pend.append((kb, b * KB + kb, [e0, e1]))
                if len(pv_pend) >= 2:
                    pkb, pvt, pes = pv_pend.pop(0)
                    for h in range(HL):
                        nc.tensor.matmul(
                            pO[h][0:65, :],
                            lhsT=v_aug[:, pvt, h * 65:(h + 1) * 65],
                            rhs=pes[h],
                            start=(pkb == 0), stop=(pkb == KB - 1))
            for pkb, pvt, pes in pv_pend:
                for h in range(HL):
                    nc.tensor.matmul(
                        pO[h][0:65, :],
                        lhsT=v_aug[:, pvt, h * 65:(h + 1) * 65],
                        rhs=pes[h],
                        start=(pkb == 0), stop=(pkb == KB - 1))
            pv_pend = []

            # free PSUM promptly: 1/denominator via ln+exp (same ACT table),
            # copy unnormalized O to SBUF; defer the normalize+silu one chunk
            ous, rbs = [], []
            for h in range(HL):
                lsl = attd.tile([1, QW], F32, name=f"lsl{h}")
                nc.scalar.activation(out=lsl, in_=pO[h][64:65, :], func=AF.Ln)
                rsl = attd.tile([1, QW], F32, name=f"rsl{h}")
                nc.scalar.activation(out=rsl, in_=lsl, func=AF.Exp, scale=-1.0)
                rd = dramd.tile([1, QW], F32, name=f"rd{h}")
                nc.sync.dma_start(out=rd, in_=rsl)
                rb = attd.tile([64, QW], F32, name=f"rb{h}")
                nc.sync.dma_start(out=rb, in_=rd[:, :].to_broadcast([64, QW]))
                ou = attd.tile([64, QW], F32, name=f"ou{h}")
                if h == 0:
                    nc.vector.tensor_copy(out=ou, in_=pO[h][0:64, :])
                else:
                    nc.scalar.copy(out=ou, in_=pO[h][0:64, :])
                ous.append(ou)
                rbs.append(rb)
            norm_pend.append((ci, q0, ous, rbs))
            if len(norm_pend) >= 2:
                emit_norm_chunk(norm_pend.pop(0))
            if ci >= 2:
                emit_out_chunk(ci - 2)
        for desc in norm_pend:
            emit_norm_chunk(desc)
        emit_out_chunk(len(chunks) - 2)
        emit_out_chunk(len(chunks) - 1)

    octx.close()


def make_in_maps(inputs, n_tok_per_batch, n_cores=NCORES):
    """Slice full inputs into per-core input maps (head sharding)."""
    import ml_dtypes
    bf16 = ml_dtypes.bfloat16
    x = np.ascontiguousarray(np.asarray(inputs["x"], np.float32)
                             .reshape(B * n_tok_per_batch, C))
    w_q = np.asarray(inputs["w_q"], np.float32)
    w_k = np.asarray(inputs["w_k"], np.float32)
    w_v = np.asarray(inputs["w_v"], np.float32)
    b_q = np.asarray(inputs["b_q"], np.float32)
    b_k = np.asarray(inputs["b_k"], np.float32)
    b_v = np.asarray(inputs["b_v"], np.float32)
    g_q = np.asarray(inputs["g_q"], np.float32)
    be_q = np.asarray(inputs["be_q"], np.float32)
    g_k = np.asarray(inputs["g_k"], np.float32)
    be_k = np.asarray(inputs["be_k"], np.float32)
    w_o = np.asarray(inputs["w_o"], np.float32)

    scale = float(INNER) ** -0.5
    in_maps = []
    for c in range(n_cores):
        cols = slice(c * CL, (c + 1) * CL)
        w_all = np.ascontiguousarray(
            np.concatenate([w_q[:, cols], w_k[:, cols], w_v[:, cols]],
                           axis=1)).astype(bf16)
        b_all = np.ascontiguousarray(
            np.concatenate([b_q[cols], b_k[cols], b_v[cols]])[None, :])
        gbe = np.ascontiguousarray(
            np.stack([g_q[cols] * scale, be_q[cols] * scale,
                      g_k[cols], be_k[cols]], axis=1))
        w_o_c = np.ascontiguousarray(w_o[cols, :]).astype(bf16)
        in_maps.append({
            "x": x, "w_all": w_all, "b_all": b_all,
            "gbe": gbe, "w_o_loc": w_o_c,
        })
    return in_maps


def combine_outputs(out_ts, inputs, n_tok_per_batch):
    b_o = np.asarray(inputs["b_o"], np.float32)
    acc = np.zeros_like(out_ts[0], dtype=np.float64)
    for o in out_ts:
        acc += o.astype(np.float64)
    out = acc.T.astype(np.float32) + b_o[None, :]
    return out.reshape(B, n_tok_per_batch, C).astype(np.float32)


_NC_CACHE = {}


def kernel(**inputs):
    from concourse.bass_utils import run_bass_kernel_spmd

    n_tok = np.asarray(inputs["x"]).shape[1]
    if n_tok not in _NC_CACHE:
        _NC_CACHE[n_tok] = build_bass(n_tok)
    nc = _NC_CACHE[n_tok]
    in_maps = make_in_maps(inputs, n_tok)
    res = run_bass_kernel_spmd(nc, in_maps, core_ids=list(range(NCORES)))
    out_ts = [r["out_t"] for r in res.results]
    return combine_outputs(out_ts, inputs, n_tok)


# revision 40
# speedup vs baseline: 1.6781x; 1.3051x over previous
"""Trainium2 Bass kernel for nn_Attention_71846212928150.

Self-attention block (pre-LN + silu, QKV projections, per-head attention with
q/k LayerNorms, output projection), sharded over 8 NeuronCores by heads:
core c owns heads {2c, 2c+1} = inner columns [128c, 128c+128).

v2: bf16 matmul datapath + head-interleaved attention.
  phase 1: stream x in 128-token tiles; LN (bn_stats + quake-rsqrt on DVE)
           fused with silu on ACT emitting bf16; PE-transpose (bf16);
           fused QKV matmul (bf16) into [tok, 384] PSUM; evict q/k/v bf16.
  phase 2: partial sum / sumsq of q,k over the local 128 columns;
           AllReduce [128,128] f32 stats across the 8 cores.
  phase 3: apply q/k LayerNorm in [tok, col] layout, PE-transpose to
           [col, tok] bf16, apply gain/bias (inner**-0.5 folded into q gain).
  phase 4: per (batch, q-chunk): both heads interleaved. S matmuls have K=64
           and per-head base partitions 0/64, so the two heads' S matmuls run
           CONCURRENTLY on disjoint PE row-groups (auto tile_position).
           exp on ACT (no max subtraction; |scores| small by construction)
           emits bf16; PV matmul with ones-column in V accumulates the
           softmax denominator in PSUM row 64.
  phase 4b: batched denominator reciprocal: gather all [1,512] denominator
           rows to DRAM, reload as [128,64], one DVE reciprocal, scatter-
           broadcast back to [128, T] via DMA; normalize + silu in two
           full-width passes.
  phase 5: output projection (bf16) -> out^T [1024, 4096] f32 partial sums,
           host adds the 8 partials, transposes, adds b_o.
"""

import numpy as np

import concourse.bass as bass
import concourse.mybir as mybir
import concourse.tile as tile
from concourse.masks import make_identity

F32 = mybir.dt.float32
F32R = mybir.dt.float32r
BF16 = mybir.dt.bfloat16
I32 = mybir.dt.int32
AF = mybir.ActivationFunctionType
ALU = mybir.AluOpType
AX = mybir.AxisListType

B = 2
C = 1024
H = 16
DH = 64
INNER = H * DH
NCORES = 8
HL = H // NCORES          # 2 heads per core
CL = HL * DH              # 128 local inner columns
QKV = 3 * CL              # 384
NW = QKV + 4              # + per-token stat columns: S1q, S1k, BDq, BDk
KT = C // 128             # 8 contraction tiles over C
EPS = 1e-5
MAGIC = 0x5F3759DF
QW = 512                  # attention q-chunk width


def _quake_rsqrt(nc, pool, vpe, shape, suffix=""):
    """rstd = 1/sqrt(vpe) entirely on DVE (fp32-exact after 3 Newton steps)."""
    y = pool.tile(list(shape), F32, name=f"qk_y{suffix}")
    t2 = pool.tile(list(shape), F32, name=f"qk_t2{suffix}")
    nc.vector.tensor_scalar(
        out=y.bitcast(I32), in0=vpe.bitcast(I32), scalar1=1, scalar2=None,
        op0=ALU.logical_shift_right)
    nc.vector.tensor_scalar(
        out=y.bitcast(I32), in0=y.bitcast(I32), scalar1=-1, scalar2=MAGIC,
        op0=ALU.mult, op1=ALU.add)
    for _ in range(3):
        nc.vector.tensor_tensor(out=t2, in0=y, in1=y, op=ALU.mult)
        nc.vector.tensor_tensor(out=t2, in0=t2, in1=vpe, op=ALU.mult)
        nc.vector.tensor_scalar(out=t2, in0=t2, scalar1=-0.5, scalar2=1.5,
                                op0=ALU.mult, op1=ALU.add)
        nc.vector.tensor_tensor(out=y, in0=y, in1=t2, op=ALU.mult)
    return y


def _quake_rsqrt2(nc, pool, vpe, shape, suffix=""):
    """Two-iteration variant (~4e-6 rel err) for the latency-critical x path."""
    y = pool.tile(list(shape), F32, name=f"qj_y{suffix}")
    t2 = pool.tile(list(shape), F32, name=f"qj_t2{suffix}")
    nc.vector.tensor_scalar(
        out=y.bitcast(I32), in0=vpe.bitcast(I32), scalar1=1, scalar2=None,
        op0=ALU.logical_shift_right)
    nc.vector.tensor_scalar(
        out=y.bitcast(I32), in0=y.bitcast(I32), scalar1=-1, scalar2=MAGIC,
        op0=ALU.mult, op1=ALU.add)
    for _ in range(2):
        nc.vector.tensor_tensor(out=t2, in0=y, in1=y, op=ALU.mult)
        nc.vector.tensor_tensor(out=t2, in0=t2, in1=vpe, op=ALU.mult)
        nc.vector.tensor_scalar(out=t2, in0=t2, scalar1=-0.5, scalar2=1.5,
                                op0=ALU.mult, op1=ALU.add)
        nc.vector.tensor_tensor(out=y, in0=y, in1=t2, op=ALU.mult)
    return y


def _fixup_module(nc):
    """Adapt Tile-emitted BIR to this container's walrus build.

    1. The tail `EVENT_SEMAPHORE_RANGE_CLEAR` InstISA (opcode 176) is not
       understood by this walrus' birverifier. Replace it with one
       EventSemaphore sem-write-0 per semaphore in the cleared range
       (functionally equivalent, re-execution stays safe).
    2. Drain instructions carrying more than one semaphore wait fail codegen
       ("Too many sync wait commands"). Hoist the extra waits into standalone
       EventSemaphore wait instructions just before the drain.
    """
    for f in nc.m.functions:
        for bb in f.blocks:
            newlist = []
            changed = False
            for ins in bb.instructions:
                tn = type(ins).__name__
                if tn == "InstISA" and getattr(ins, "isa_opcode", None) == 176:
                    ad = ins.ant_dict or {}
                    first = ad.get("range_first")
                    last = ad.get("range_last")
                    if first is not None and last is not None:
                        si = ins.sync_info
                        sems = list(range(first, last + 1))
                        for k, sem in enumerate(sems):
                            ev = mybir.InstEventSemaphore(
                                name=f"{ins.name}-clr{k}", engine=ins.engine,
                                ins=[], outs=[])
                            upd = mybir.SyncUpdate(
                                sync_type="semaphore", id=sem,
                                update_mode="sem-wr-imm", update_value=0)
                            on_wait = (list(si.on_wait)
                                       if (k == 0 and si is not None and si.on_wait)
                                       else [])
                            ev.sync_info = mybir.SyncInfo(
                                on_wait=on_wait, on_update=[upd])
                            newlist.append(ev)
                        if si is not None and si.on_update:
                            evf = mybir.InstEventSemaphore(
                                name=f"{ins.name}-clrf", engine=ins.engine,
                                ins=[], outs=[])
                            evf.sync_info = mybir.SyncInfo(
                                on_wait=[], on_update=list(si.on_update))
                            newlist.append(evf)
                    changed = True
                    continue
                si = ins.sync_info
                if (si is not None and si.on_wait is not None
                        and len(si.on_wait) > 1):
                    waits = list(si.on_wait)
                    for i, w in enumerate(waits[1:]):
                        ev = mybir.InstEventSemaphore(
                            name=f"{ins.name}-hw{i}", engine=ins.engine,
                            ins=[], outs=[])
                        ev.sync_info = mybir.SyncInfo(on_wait=[w], on_update=[])
                        newlist.append(ev)
                    si.on_wait = [waits[0]]
                    ins.sync_info = si
                    changed = True
                newlist.append(ins)
            if changed:
                bb.instructions = newlist
    return nc


def build_bass(n_tok_per_batch, n_cores=NCORES, bv_nonzero=True):
    N = n_tok_per_batch
    T = B * N
    NT = T // 128             # token tiles
    KB = N // 128             # key tiles per batch
    QC = N // QW              # q chunks per batch
    OTC = max(1, T // 512)    # out-proj token chunks
    OTW = min(512, T)

    nc = bass.Bass(trn_type="TRN2", num_devices=n_cores)

    x = nc.dram_tensor("x", [T, C], F32, kind="ExternalInput")
    w_all = nc.dram_tensor("w_all", [C, NW], BF16, kind="ExternalInput")
    b_all = nc.dram_tensor("b_all", [1, QKV], F32, kind="ExternalInput")
    gbe = nc.dram_tensor("gbe", [128, 8], F32, kind="ExternalInput")
    w_o_loc = nc.dram_tensor("w_o_loc", [CL, C], BF16, kind="ExternalInput")
    out_t = nc.dram_tensor("out_t", [C, T], BF16, kind="ExternalOutput")

    with tile.TileContext(nc) as tc:
        _body(tc, x, w_all, b_all, gbe, w_o_loc, out_t,
              N=N, T=T, NT=NT, KB=KB, QC=QC, OTC=OTC, OTW=OTW,
              n_cores=n_cores, bv_nonzero=bv_nonzero)
    return _fixup_module(nc)


def _body(tc, x, w_all, b_all, gbe, w_o_loc, out_t,
          N, T, NT, KB, QC, OTC, OTW, n_cores, bv_nonzero):
    nc = tc.nc

    from contextlib import ExitStack
    octx = ExitStack()
    persist = octx.enter_context(tc.tile_pool(name="persist", bufs=1))

    ident = persist.tile([128, 128], BF16)
    make_identity(nc, ident)

    w_all_sb = persist.tile([128, KT, NW], BF16)
    for kt in range(KT):
        nc.sync.dma_start(out=w_all_sb[:, kt, :],
                          in_=w_all[kt * 128:(kt + 1) * 128, :])
    b_sb = persist.tile([128, QKV], F32)
    nc.sync.dma_start(out=b_sb, in_=b_all.ap().to_broadcast([128, QKV]))
    gbe_sb = persist.tile([128, 8], F32)
    nc.sync.dma_start(out=gbe_sb, in_=gbe[:, :])
    w_o_sb = persist.tile([128, C], BF16)
    nc.sync.dma_start(out=w_o_sb, in_=w_o_loc[:, :])

    qT = persist.tile([128, T], BF16)      # [local col, token]
    kTt = persist.tile([128, T], BF16)
    v_aug = persist.tile([128, NT, 130], BF16)  # [tok%128, tile, head-block]
    q_pre = persist.tile([128, NT, 128], BF16)  # [tok%128, tile, local col]
    k_pre = persist.tile([128, NT, 128], BF16)
    stats = persist.tile([128, 6, NT], F32)
    stats_all = persist.tile([128, 6, NT], F32)
    onrm = persist.tile([128, T], F32)     # normalized attention out^T
    rbc = [persist.tile([64, T], F32, name=f"rbc{h}") for h in range(HL)]
    siluo = persist.tile([128, T], BF16)

    ones_col = persist.tile([128, NT], BF16)
    nc.vector.memset(ones_col, 1.0)
    nc.vector.tensor_copy(out=v_aug[:, :, 64:65], in_=ones_col)
    nc.vector.tensor_copy(out=v_aug[:, :, 129:130], in_=ones_col)

    HNT = NT // 2
    dpool = octx.enter_context(tc.tile_pool(name="dramcc", bufs=1,
                                            space="DRAM"))
    cc_ins = [dpool.tile([128, 6, HNT], F32, name=f"cc_in{h}")
              for h in range(2)]
    cc_outs = [dpool.tile([128, 6, HNT], F32, name=f"cc_out{h}",
                          addr_space="Shared") for h in range(2)]

    def emit_cc(half):
        sl = slice(half * HNT, (half + 1) * HNT)
        nc.sync.dma_start(out=cc_ins[half], in_=stats[:, :, sl])
        nc.gpsimd.collective_compute(
            "AllReduce", ALU.add,
            replica_groups=[list(range(n_cores))],
            ins=[cc_ins[half].opt()], outs=[cc_outs[half].opt()])
        nc.sync.dma_start(out=stats_all[:, :, sl], in_=cc_outs[half])

    # ---------------- phase 1: x-side LN+silu, transpose, QKV ----------------
    # Per-token q/k stats come out of the same matmul (4 extra weight
    # columns: sum-of-w and b-dot-w) + per-tile ACT Square accumulation,
    # so phase 2 is just the AllReduce.  The QKV matmul for tile t-1 is
    # emitted after tile t's transposes (software pipelining: PE never
    # waits on the xsT eviction).
    GB = 4  # token tiles per group
    with tc.tile_pool(name="ph1", bufs=2) as ph1, \
         tc.tile_pool(name="ph1x", bufs=2) as ph1x, \
         tc.tile_pool(name="ph1s", bufs=4) as ph1s, \
         tc.tile_pool(name="ph1p", bufs=2, space="PSUM") as ph1p, \
         tc.tile_pool(name="ph1q", bufs=3, space="PSUM") as ph1q:

        def emit_qkv(tt, xsT):
            pqkv = ph1q.tile([128, 512], F32, name="pqkv")
            for kt in range(KT):
                nc.tensor.matmul(
                    pqkv[:, 0:NW],
                    lhsT=xsT[:, kt * 128:(kt + 1) * 128],
                    rhs=w_all_sb[:, kt, :],
                    start=(kt == 0), stop=(kt == KT - 1))
            # evictions: q/k on DVE (with bias), v on ACT, stats accumulate
            nc.vector.scalar_tensor_tensor(
                out=q_pre[:, tt, :], in0=pqkv[:, 0:128], scalar=1.0,
                in1=b_sb[:, 0:128], op0=ALU.mult, op1=ALU.add)
            nc.vector.scalar_tensor_tensor(
                out=k_pre[:, tt, :], in0=pqkv[:, 128:256], scalar=1.0,
                in1=b_sb[:, 128:256], op0=ALU.mult, op1=ALU.add)
            if bv_nonzero:
                nc.vector.scalar_tensor_tensor(
                    out=v_aug[:, tt, :].rearrange("p (h e) -> p h e", e=65)[:, :, 0:64],
                    in0=pqkv[:, 256:384].rearrange("p (h e) -> p h e", e=64),
                    scalar=1.0,
                    in1=b_sb[:, 256:384].rearrange("p (h e) -> p h e", e=64),
                    op0=ALU.mult, op1=ALU.add)
            else:
                nc.scalar.copy(
                    out=v_aug[:, tt, :].rearrange("p (h e) -> p h e", e=65)[:, :, 0:64],
                    in_=pqkv[:, 256:384].rearrange("p (h e) -> p h e", e=64))
            scr = ph1s.tile([128, 128], BF16, name="scr")
            nc.scalar.activation(
                out=scr, in_=pqkv[:, 0:128], func=AF.Square,
                accum_out=stats[:, 0, tt:tt + 1])
            nc.scalar.activation(
                out=scr, in_=pqkv[:, 128:256], func=AF.Square,
                accum_out=stats[:, 1, tt:tt + 1])
            nc.vector.tensor_copy(out=stats[:, 2:6, tt],
                                  in_=pqkv[:, QKV:NW])

        prev = None
        def emit_qkv_hooked(tt, xsT):
            emit_qkv(tt, xsT)
            if tt == NT // 2 - 1:
                emit_cc(0)
        for g in range(NT // GB):
            xg = ph1x.tile([128, GB, C], F32, name="xg")
            nc.sync.dma_start(
                out=xg,
                in_=x[g * GB * 128:(g + 1) * GB * 128, :].rearrange(
                    "(t p) c -> p t c", p=128))

            stats6 = ph1s.tile([128, GB, 2, 6], F32, name="stats6")
            for t in range(GB):
                for h2 in range(2):
                    nc.vector.bn_stats(out=stats6[:, t, h2, :],
                                       in_=xg[:, t, h2 * 512:(h2 + 1) * 512])
            mv = ph1s.tile([128, GB, 2], F32, name="mv")
            for t in range(GB):
                nc.vector.bn_aggr(out=mv[:, t, :], in_=stats6[:, t, :, :])

            vpe = ph1s.tile([128, GB, 1], F32, name="vpe")
            nc.vector.tensor_scalar(out=vpe, in0=mv[:, :, 1:2], scalar1=EPS,
                                    scalar2=None, op0=ALU.add)
            rstd = _quake_rsqrt2(nc, ph1s, vpe, (128, GB, 1))
            nmr = ph1s.tile([128, GB, 1], F32, name="nmr")
            nc.vector.tensor_tensor(out=nmr, in0=mv[:, :, 0:1], in1=rstd,
                                    op=ALU.mult)
            nc.vector.tensor_scalar(out=nmr, in0=nmr, scalar1=-1.0,
                                    scalar2=None, op0=ALU.mult)

            # silu(LN(x)) -> bf16
            xs = ph1.tile([128, GB, C], BF16, name="xs")
            for t in range(GB):
                nc.scalar.activation(out=xs[:, t, :], in_=xg[:, t, :],
                                     func=AF.Silu,
                                     bias=nmr[:, t, :],
                                     scale=rstd[:, t, :])

            for t in range(GB):
                tt = g * GB + t
                pxT = ph1p.tile([128, 1024], BF16, name="pxT")
                for j in range(KT):
                    nc.tensor.transpose(pxT[:, j * 128:(j + 1) * 128],
                                        xs[:, t, j * 128:(j + 1) * 128],
                                        ident)
                xsT = ph1.tile([128, 1024], BF16, name="xsT")
                nc.vector.tensor_copy(out=xsT, in_=pxT)
                if prev is not None:
                    emit_qkv_hooked(*prev)
                prev = (tt, xsT)
        emit_qkv_hooked(*prev)
        emit_cc(1)

    # -------- phase 2+3: per-half stats math, LN apply, transpose --------
    # the half-0 collective ran under the second half of phase 1; half-1's
    # collective overlaps half-0's math/apply/transpose work here
    with tc.tile_pool(name="ph2", bufs=1) as ph2, \
         tc.tile_pool(name="ph3", bufs=8) as ph3, \
         tc.tile_pool(name="ph3p", bufs=4, space="PSUM") as ph3p:
        def half_stats(which, half):
            # stats rows: 0 S2q, 1 S2k, 2 S1q, 3 S1k, 4 BDq, 5 BDk
            # gbe cols: 4 c1q, 5 c1k, 6 c2q, 7 c2k (c1 = sum b, c2 = sum b^2)
            sl = slice(half * HNT, (half + 1) * HNT)
            s_ssq = stats_all[:, 0 + which, sl]
            s_sum = stats_all[:, 2 + which, sl]
            s_bd = stats_all[:, 4 + which, sl]
            sfx = f"_{which}_{half}"
            m = ph2.tile([128, HNT], F32, name=f"m{sfx}")
            nc.vector.tensor_scalar(out=m, in0=s_sum,
                                    scalar1=gbe_sb[:, 4 + which:5 + which],
                                    scalar2=1.0 / INNER,
                                    op0=ALU.add, op1=ALU.mult)
            msq = ph2.tile([128, HNT], F32, name=f"msq{sfx}")
            nc.vector.tensor_scalar(out=msq, in0=s_bd, scalar1=2.0,
                                    scalar2=None, op0=ALU.mult)
            nc.vector.tensor_tensor(out=msq, in0=msq, in1=s_ssq, op=ALU.add)
            nc.vector.tensor_scalar(out=msq, in0=msq,
                                    scalar1=gbe_sb[:, 6 + which:7 + which],
                                    scalar2=1.0 / INNER,
                                    op0=ALU.add, op1=ALU.mult)
            tmp = ph2.tile([128, HNT], F32, name=f"tmp{sfx}")
            nc.vector.tensor_tensor(out=tmp, in0=m, in1=m, op=ALU.mult)
            nc.vector.tensor_tensor(out=tmp, in0=msq, in1=tmp, op=ALU.subtract)
            nc.vector.tensor_scalar(out=tmp, in0=tmp, scalar1=EPS,
                                    scalar2=None, op0=ALU.add)
            rstd = _quake_rsqrt(nc, ph2, tmp, (128, HNT), suffix=sfx)
            nmr = ph2.tile([128, HNT], F32, name=f"nmr{sfx}")
            nc.vector.tensor_tensor(out=nmr, in0=m, in1=rstd, op=ALU.mult)
            nc.vector.tensor_scalar(out=nmr, in0=nmr, scalar1=-1.0,
                                    scalar2=None, op0=ALU.mult)
            return m, rstd, nmr

        def emit_T(which, tt, qn, dst, gcol):
            pq = ph3p.tile([128, 128], BF16, name="pq")
            nc.tensor.transpose(pq, qn, ident)
            if which == 0:
                nc.scalar.activation(
                    out=dst[:, tt * 128:(tt + 1) * 128], in_=pq,
                    func=AF.Identity,
                    bias=gbe_sb[:, gcol + 1:gcol + 2],
                    scale=gbe_sb[:, gcol:gcol + 1])
            else:
                nc.vector.tensor_scalar(
                    out=dst[:, tt * 128:(tt + 1) * 128], in0=pq,
                    scalar1=gbe_sb[:, gcol:gcol + 1],
                    scalar2=gbe_sb[:, gcol + 1:gcol + 2],
                    op0=ALU.mult, op1=ALU.add)

        prevT = None
        for half in range(2):
            for which, (pre, dst, gcol) in enumerate(
                    [(q_pre, qT, 0), (k_pre, kTt, 2)]):
                m, rstd, nmr = half_stats(which, half)
                for tt in range(half * HNT, (half + 1) * HNT):
                    ti = tt - half * HNT
                    qn = ph3.tile([128, 128], BF16, name="qn")
                    if which == 1:
                        nc.scalar.activation(
                            out=qn, in_=pre[:, tt, :], func=AF.Identity,
                            bias=nmr[:, ti:ti + 1],
                            scale=rstd[:, ti:ti + 1])
                    else:
                        nc.vector.tensor_scalar(
                            out=qn, in0=pre[:, tt, :],
                            scalar1=m[:, ti:ti + 1],
                            scalar2=rstd[:, ti:ti + 1],
                            op0=ALU.subtract, op1=ALU.mult)
                    if prevT is not None:
                        emit_T(*prevT)
                    prevT = (which, tt, qn, dst, gcol)
        emit_T(*prevT)

    # ---------------- phase 4: attention (heads interleaved) ----------------
    # Schraudolph fast-exp emitting bf16 bits directly (DVE, one op):
    LOG2E = 1.4426950408889634
    A16 = 128.0 * LOG2E
    B16 = float(0x3F80) - 486408.0 / 65536.0
    I16 = mybir.dt.int16

    with tc.tile_pool(name="att", bufs=3) as att, \
         tc.tile_pool(name="attd", bufs=5) as attd, \
         tc.tile_pool(name="dramd", bufs=2, space="DRAM") as dramd, \
         tc.tile_pool(name="attp", bufs=2, space="PSUM") as attp, \
         tc.tile_pool(name="attpo", bufs=1, space="PSUM") as attpo:

        def emit_recip_chunk(b, qc, q0):
            # this chunk's denominators: spread over 64 lanes, one DVE
            # reciprocal per head, partition-broadcast back
            for h in range(HL):
                doff = (b * HL + h) * N + qc * QW
                dg = attd.tile([64, QW // 64], F32, name=f"dg{h}")
                nc.sync.dma_start(
                    out=dg,
                    in_=d_dram[0, doff:doff + QW].rearrange(
                        "(p f) -> p f", p=64))
                rg = attd.tile([64, QW // 64], F32, name=f"rg{h}")
                nc.vector.reciprocal(out=rg, in_=dg)
                nc.sync.dma_start(
                    out=r_dram[0, doff:doff + QW].rearrange(
                        "(p f) -> p f", p=64), in_=rg)
                nc.sync.dma_start(
                    out=rbc[h][:, q0:q0 + QW],
                    in_=r_dram[:, doff:doff + QW].to_broadcast([64, QW]))

        def emit_norm_chunk(desc):
            # deferred: normalize for a finished chunk (rbc slice has landed)
            ci, q0, ous = desc
            for h in range(HL):
                nc.vector.tensor_tensor(
                    out=onrm[h * 64:(h + 1) * 64, q0:q0 + QW],
                    in0=ous[h][0:64, :],
                    in1=rbc[h][:, q0:q0 + QW], op=ALU.mult)

        chunks = [(b, qc) for b in range(B) for qc in range(QC)]
        d_dram = dramd.tile([1, B * HL * N], F32, name="d_dram")
        r_dram = dramd.tile([1, B * HL * N], F32, name="r_dram")
        norm_pend = []
        for ci, (b, qc) in enumerate(chunks):
            q0 = b * N + qc * QW
            # two PSUM banks per head: upper-half-K and lower-half-K partial
            # sums of PV, combined during the eviction
            pO = [[attpo.tile([128, QW], F32, name=f"pO{h}{u}")
                   for u in range(1)] for h in range(HL)]
            pv_pend = []
            for kb in range(KB):
                pS = [attp.tile([128, QW], F32, name=f"pS{h}")
                      for h in range(HL)]
                for h in range(HL):
                    # K=64 with base partition h*64: the two heads' S matmuls
                    # are adjacent in the PE stream and run concurrently on
                    # disjoint row-groups (auto tile_position)
                    nc.tensor.matmul(
                        pS[h],
                        lhsT=kTt[h * 64:(h + 1) * 64,
                                 b * N + kb * 128:b * N + (kb + 1) * 128],
                        rhs=qT[h * 64:(h + 1) * 64, q0:q0 + QW],
                        start=True, stop=True)
                e0 = att.tile([128, QW], BF16, name="eS0")
                nc.scalar.activation(out=e0, in_=pS[0], func=AF.Exp)
                e1 = att.tile([128, QW], BF16, name="eS1")
                nc.vector.tensor_scalar(
                    out=e1.bitcast(I16), in0=pS[1], scalar1=A16, scalar2=B16,
                    op0=ALU.mult, op1=ALU.add)
                pv_pend.append((kb, b * KB + kb, [e0, e1]))
                if len(pv_pend) >= 2:
                    pkb, pvt, pes = pv_pend.pop(0)
                    for h in range(HL):
                        nc.tensor.matmul(
                            pO[h][0][0:65, :],
                            lhsT=v_aug[:, pvt, h * 65:(h + 1) * 65],
                            rhs=pes[h],
                            start=(pkb == 0), stop=(pkb == KB - 1))
            for pkb, pvt, pes in pv_pend:
                for h in range(HL):
                    nc.tensor.matmul(
                        pO[h][0][0:65, :],
                        lhsT=v_aug[:, pvt, h * 65:(h + 1) * 65],
                        rhs=pes[h],
                        start=(pkb == 0), stop=(pkb == KB - 1))
            pv_pend = []

            # free PSUM promptly: evict O (+denominator row) to SBUF and
            # ship the denominators to DRAM; the normalize is deferred until
            # the whole batch's reciprocals come back
            ous = []
            for h in range(HL):
                ou = attd.tile([65, QW], F32, name=f"ou{h}")
                if h == 0:
                    nc.vector.tensor_copy(out=ou, in_=pO[h][0][0:65, :])
                else:
                    nc.scalar.copy(out=ou, in_=pO[h][0][0:65, :])
                doff = (b * HL + h) * N + qc * QW
                nc.sync.dma_start(out=d_dram[:, doff:doff + QW],
                                  in_=ou[64:65, :])
                ous.append(ou)
            emit_recip_chunk(b, qc, q0)
            norm_pend.append((ci, q0, ous))
            if len(norm_pend) >= 2:
                emit_norm_chunk(norm_pend.pop(0))
        for desc in norm_pend:
            emit_norm_chunk(desc)

    # ---------------- phase 5: silu + output projection ----------------
    # silu batched here (exp and silu live in different ACT tables: keep
    # exactly one table switch); silu(tk+1) emitted before tk's matmuls
    with tc.tile_pool(name="ph5", bufs=6) as ph5, \
         tc.tile_pool(name="ph5p", bufs=6, space="PSUM") as ph5p:
        with tc.tile_wait_until(0.4):
            nc.scalar.activation(out=siluo[:, 0:OTW], in_=onrm[:, 0:OTW],
                                 func=AF.Silu)
        for tk in range(OTC):
            if tk + 1 < OTC:
                with tc.tile_wait_until(0.4):
                    nc.scalar.activation(
                        out=siluo[:, (tk + 1) * OTW:(tk + 2) * OTW],
                        in_=onrm[:, (tk + 1) * OTW:(tk + 2) * OTW],
                        func=AF.Silu)
            for ct in range(KT):
                po = ph5p.tile([128, OTW], F32, name="po")
                nc.tensor.matmul(
                    po,
                    lhsT=w_o_sb[:, ct * 128:(ct + 1) * 128],
                    rhs=siluo[:, tk * OTW:(tk + 1) * OTW],
                    start=True, stop=True)
                ev = ph5.tile([128, OTW], BF16, name="ev")
                if ct % 2 == 0:
                    nc.vector.tensor_copy(out=ev, in_=po)
                else:
                    nc.scalar.copy(out=ev, in_=po)
                nc.sync.dma_start(
                    out=out_t[ct * 128:(ct + 1) * 128,
                              tk * OTW:(tk + 1) * OTW],
                    in_=ev)

    octx.close()


def make_in_maps(inputs, n_tok_per_batch, n_cores=NCORES):
    """Slice full inputs into per-core input maps (head sharding)."""
    import ml_dtypes
    bf16 = ml_dtypes.bfloat16
    x = np.ascontiguousarray(np.asarray(inputs["x"], np.float32)
                             .reshape(B * n_tok_per_batch, C))
    w_q = np.asarray(inputs["w_q"], np.float32)
    w_k = np.asarray(inputs["w_k"], np.float32)
    w_v = np.asarray(inputs["w_v"], np.float32)
    b_q = np.asarray(inputs["b_q"], np.float32)
    b_k = np.asarray(inputs["b_k"], np.float32)
    b_v = np.asarray(inputs["b_v"], np.float32)
    g_q = np.asarray(inputs["g_q"], np.float32)
    be_q = np.asarray(inputs["be_q"], np.float32)
    g_k = np.asarray(inputs["g_k"], np.float32)
    be_k = np.asarray(inputs["be_k"], np.float32)
    w_o = np.asarray(inputs["w_o"], np.float32)

    scale = float(INNER) ** -0.5
    c1q = float(b_q.sum())
    c1k = float(b_k.sum())
    c2q = float((b_q.astype(np.float64) ** 2).sum())
    c2k = float((b_k.astype(np.float64) ** 2).sum())
    in_maps = []
    for c in range(n_cores):
        cols = slice(c * CL, (c + 1) * CL)
        # stat columns: sum of local w columns (per-token raw sum) and
        # b-dot-w (per-token sum of b*raw, for the bias sumsq correction)
        ws_q = w_q[:, cols].astype(np.float64).sum(axis=1)
        ws_k = w_k[:, cols].astype(np.float64).sum(axis=1)
        bd_q = w_q[:, cols].astype(np.float64) @ b_q[cols].astype(np.float64)
        bd_k = w_k[:, cols].astype(np.float64) @ b_k[cols].astype(np.float64)
        w_all = np.ascontiguousarray(
            np.concatenate([w_q[:, cols], w_k[:, cols], w_v[:, cols],
                            ws_q[:, None].astype(np.float32),
                            ws_k[:, None].astype(np.float32),
                            bd_q[:, None].astype(np.float32),
                            bd_k[:, None].astype(np.float32)],
                           axis=1)).astype(bf16)
        b_all = np.ascontiguousarray(
            np.concatenate([b_q[cols], b_k[cols], b_v[cols]])[None, :])
        gbe = np.ascontiguousarray(np.stack(
            [g_q[cols] * scale, be_q[cols] * scale,
             g_k[cols], be_k[cols],
             np.full(CL, c1q, np.float32), np.full(CL, c1k, np.float32),
             np.full(CL, c2q, np.float32), np.full(CL, c2k, np.float32)],
            axis=1))
        w_o_c = np.ascontiguousarray(w_o[cols, :]).astype(bf16)
        in_maps.append({
            "x": x, "w_all": w_all, "b_all": b_all,
            "gbe": gbe, "w_o_loc": w_o_c,
        })
    return in_maps


def combine_outputs(out_ts, inputs, n_tok_per_batch):
    b_o = np.asarray(inputs["b_o"], np.float32)
    acc = np.zeros_like(out_ts[0], dtype=np.float64)
    for o in out_ts:
        acc += o.astype(np.float64)
    out = acc.T.astype(np.float32) + b_o[None, :]
    return out.reshape(B, n_tok_per_batch, C).astype(np.float32)


_NC_CACHE = {}


def kernel(**inputs):
    from concourse.bass_utils import run_bass_kernel_spmd

    n_tok = np.asarray(inputs["x"]).shape[1]
    bv = bool(np.any(np.asarray(inputs["b_v"])))
    key = (n_tok, bv)
    if key not in _NC_CACHE:
        _NC_CACHE[key] = build_bass(n_tok, bv_nonzero=bv)
    nc = _NC_CACHE[key]
    in_maps = make_in_maps(inputs, n_tok)
    res = run_bass_kernel_spmd(nc, in_maps, core_ids=list(range(NCORES)))
    out_ts = [r["out_t"] for r in res.results]
    return combine_outputs(out_ts, inputs, n_tok)
